# revision 9
# baseline (speedup 1.0000x reference)
"""Trainium2 Bass kernel for nn_MoEClassifier (moe_routing).

Model (per sample):
  x[16,5] -> flat 80 -> fc1(80->64) gelu -> fc2(64->64) gelu -> LN -> h
  u = user_table[user_id]  (16)
  gate: g_e = sum_r (h @ gU[e])_r * (u @ gV[e])_r + gb_e ; top-2 softmax -> w
  experts (dense): z_e = gelu(h @ e_w1[e] + e_b1[e]); LN(z); lpe = z @ e_w2[e] + e_b2
  logits = sum_e w_e * lpe_e   (10 classes)

Strategy: pure data-parallel across 8 NeuronCores (batch 131072 -> 16384/core).
On-chip layout is feature-major ([feature partitions, batch free]).  Per-sample
scalar math (LN rsqrt, top-2 gate) runs batch-major via PE transposes.
Expert LN is folded algebraically into the expert fc2 / combine stage:
  lpe = rs*( (z*g)@w2 - mu*(g@w2) ) + (beta@w2 + b2)
  logits = sum_e ws_e*A_e - sum_e wsm_e*gw2[e] + sum_e w_e*const[e]
with ws = w*rs, wsm = w*rs*mu.
"""
import sys, os

for _p in ("/opt/trn_rl_repo",):
    if _p not in sys.path:
        sys.path.insert(0, _p)

import numpy as np
from contextlib import ExitStack

import concourse.bass as bass
import concourse.tile as tile
from concourse import bacc, mybir

F32 = mybir.dt.float32
F32R = mybir.dt.float32r
I16 = mybir.dt.int16
I32 = mybir.dt.int32
AF = mybir.ActivationFunctionType
ALU = mybir.AluOpType

# Model dims (hardcoded per problem spec)
B = 131072
NCORES = 8
B_CORE = B // NCORES
IN_F = 80
EMB = 64
UDIM = 16
E = 16
RANK = 8
NCLS = 10
NUSERS = 1000
EPS_LN = 1e-5
TN = 512          # streaming tile width (one PSUM bank of fp32)
NCH = TN // 128   # 128-chunks per tile

# expert row order in the per-sample scalar block (see mu/m2 copy layout)
PERM = list(range(16))  # natural order (stats extraction preserves it)

MMDT_DEFAULT = "f32r"   # "f32" (exact, 4 cyc/row) or "f32r" (~2e-4 rel, 1 cyc/row)


def _bc(ap, n):
    """broadcast the (size-1) innermost dim of an AP to n via stride 0"""
    return ap.to_broadcast(list(ap.shape[:-1]) + [n])


def build_program(b_core=B_CORE, mmdt=MMDT_DEFAULT):
    MMDT = F32R if mmdt == "f32r" else F32
    ntiles = b_core // TN
    nc = bacc.Bacc("TRN2", target_bir_lowering=False, debug=False,
                   num_devices=NCORES)

    # ---------------- DRAM I/O ----------------
    d_x = nc.dram_tensor("x", [ntiles, NCH, 128, IN_F], F32, kind="ExternalInput")
    d_ids = nc.dram_tensor("ids", [ntiles, 128, 4], I16, kind="ExternalInput")
    d_out = nc.dram_tensor("out", [ntiles, NCH, 128, NCLS], F32, kind="ExternalOutput")

    def cin(name, shape, dt=F32):
        return nc.dram_tensor(name, shape, dt, kind="ExternalInput")

    d_ident = cin("ident", [128, 128])
    d_wbb1 = cin("wbb1", [IN_F, EMB], MMDT)
    d_wbb2 = cin("wbb2", [EMB, EMB], MMDT)
    d_b1 = cin("b1c", [EMB, 1])
    d_b2 = cin("b2c", [EMB, 1])
    d_beta = cin("betac", [EMB, 1])
    d_stat64 = cin("stat64", [128, 32], MMDT)
    d_stl = cin("st_lhs", [2, 128], MMDT)
    d_wgU = cin("wgU", [EMB, 128], MMDT)
    d_wgV = cin("wgV", [UDIM, 128], MMDT)
    d_gsum = cin("gsum_lhs", [128, E], MMDT)
    d_gb = cin("gb_col", [E, 1])
    d_we1 = cin("we1", [128, 4, 128], MMDT)
    d_eb1 = cin("eb1", [128, 8])
    d_we2 = cin("we2", [128, 8, 32], MMDT)
    d_wsb = cin("wsb_lhs", [48, 2, 128], MMDT)
    d_msum = cin("msum_lhs", [128, NCLS], MMDT)
    d_gw2c = cin("gw2c_lhs", [2 * E, NCLS], MMDT)
    d_ut = cin("ut_rep", [128, NUSERS])

    with tile.TileContext(nc) as tc, ExitStack() as ctx:
        cpool = ctx.enter_context(tc.tile_pool(name="consts", bufs=1))
        p_in = ctx.enter_context(tc.tile_pool(name="inp", bufs=3))
        p_w = ctx.enter_context(tc.tile_pool(name="work", bufs=2))
        p_sc = ctx.enter_context(tc.tile_pool(name="scal", bufs=2))
        p_z = ctx.enter_context(tc.tile_pool(name="zsb", bufs=9))
        p_z2 = ctx.enter_context(tc.tile_pool(name="z2sb", bufs=3))
        p_out = ctx.enter_context(tc.tile_pool(name="osb", bufs=3))
        ps_m = ctx.enter_context(tc.tile_pool(name="psm", bufs=4, space="PSUM"))
        ps_z = ctx.enter_context(tc.tile_pool(name="psz", bufs=2, space="PSUM"))
        ps_f = ctx.enter_context(tc.tile_pool(name="psf", bufs=2, space="PSUM"))

        # ---------------- constants to SBUF ----------------
        c = {}
        for name, d, shape, dt in [
            ("ident", d_ident, [128, 128], F32),
            ("wbb1", d_wbb1, [IN_F, EMB], MMDT),
            ("wbb2", d_wbb2, [EMB, EMB], MMDT),
            ("b1", d_b1, [EMB, 1], F32),
            ("b2", d_b2, [EMB, 1], F32),
            ("beta", d_beta, [EMB, 1], F32),
            ("stat64", d_stat64, [128, 32], MMDT),
            ("stl", d_stl, [2, 128], MMDT),
            ("wgU", d_wgU, [EMB, 128], MMDT),
            ("wgV", d_wgV, [UDIM, 128], MMDT),
            ("gsum", d_gsum, [128, E], MMDT),
            ("gb", d_gb, [E, 1], F32),
            ("we1", d_we1, [128, 4, 128], MMDT),
            ("eb1", d_eb1, [128, 8], F32),
            ("we2", d_we2, [128, 8, 32], MMDT),
            ("wsb", d_wsb, [48, 2, 128], MMDT),
            ("msum", d_msum, [128, NCLS], MMDT),
            ("gw2c", d_gw2c, [2 * E, NCLS], MMDT),
            ("ut", d_ut, [128, NUSERS], F32),
        ]:
            t = cpool.tile(shape, dt, tag=name)
            nc.sync.dma_start(t[:], d.ap())
            c[name] = t

        ident = c["ident"]

        for it in range(ntiles):
            # ---------- load x (batch-major) and ids ----------
            x_bm = p_in.tile([128, NCH, IN_F], F32, tag="x_bm")
            nc.sync.dma_start(x_bm[:], d_x.ap()[it].rearrange("c p f -> p c f"))
            ids_t = p_in.tile([128, 4], I16, tag="ids_t")
            nc.sync.dma_start(ids_t[:], d_ids.ap()[it])

            # ---------- transpose x to feature-major ----------
            xT = ps_m.tile([IN_F, TN], F32, tag="psm")
            for ch in range(NCH):
                nc.tensor.transpose(xT[:, 128 * ch:128 * (ch + 1)],
                                    x_bm[:, ch, :], ident[:])
            x_fm = p_w.tile([IN_F, TN], MMDT, tag="x_fm")
            nc.vector.tensor_copy(x_fm[:], xT[:])

            # ---------- backbone ----------
            ps1 = ps_m.tile([EMB, TN], F32, tag="psm")
            nc.tensor.matmul(ps1[:], c["wbb1"][:], x_fm[:], start=True, stop=True)
            h1 = p_w.tile([EMB, TN], MMDT, tag="h1")
            nc.scalar.activation(h1[:], ps1[:], AF.Gelu, bias=c["b1"][:])

            ps2 = ps_m.tile([EMB, TN], F32, tag="psm")
            nc.tensor.matmul(ps2[:], c["wbb2"][:], h1[:], start=True, stop=True)
            h2s = p_w.tile([128, TN], MMDT, tag="h2s")   # rows 0-63 h2, 64-127 h2^2
            nc.scalar.activation(h2s[0:EMB, :], ps2[:], AF.Gelu, bias=c["b2"][:])
            nc.scalar.activation(h2s[EMB:128, :], h2s[0:EMB, :], AF.Square)

            psb = ps_m.tile([2, TN], F32, tag="psm")     # mean(h2), mean(h2^2)
            nc.tensor.matmul(psb[:], c["stat64"][:, 0:2], h2s[:], start=True, stop=True)
            stats_bb = p_sc.tile([2, TN], F32, tag="stats_bb")
            nc.vector.tensor_copy(stats_bb[:], psb[:])

            # ---------- pass A: bb LN scalars (batch-major) ----------
            psA = ps_m.tile([128, NCH, 2], F32, tag="psm")
            for ch in range(NCH):
                nc.tensor.transpose(psA[:, ch, :], stats_bb[:, 128 * ch:128 * (ch + 1)],
                                    ident[0:2, 0:2])
            # var = (m2 + eps) - mu^2 ; rs = rsqrt(var) ; p = mu*rs
            sA = p_sc.tile([128, NCH, 2], F32, tag="sA")
            nc.vector.tensor_copy(sA[:], psA[:])
            tmpA = p_sc.tile([128, NCH], F32, tag="tmpA")
            nc.vector.tensor_tensor(tmpA[:], sA[:, :, 0], sA[:, :, 0], op=ALU.mult)
            vA = p_sc.tile([128, NCH], F32, tag="vA")
            nc.vector.scalar_tensor_tensor(vA[:], sA[:, :, 1], EPS_LN, tmpA[:],
                                           op0=ALU.add, op1=ALU.subtract)
            backA = p_sc.tile([128, NCH, 2], F32, tag="backA")
            rsA = backA[:, :, 0]
            _newton_rsqrt(nc, p_sc, vA[:], rsA, [128, NCH], "nA")
            nc.vector.tensor_tensor(backA[:, :, 1], rsA, sA[:, :, 0], op=ALU.mult)

            psBA = ps_m.tile([2, TN], F32, tag="psm")
            for ch in range(NCH):
                nc.tensor.transpose(psBA[:, 128 * ch:128 * (ch + 1)],
                                    backA[:, ch, :], ident[:])
            stf = p_sc.tile([2, TN], MMDT, tag="stf")
            nc.vector.tensor_copy(stf[:], psBA[:])

            # ---------- h = h2*S + (beta + T') ----------
            stp = ps_m.tile([128, TN], F32, tag="psm")
            nc.tensor.matmul(stp[:], c["stl"][:], stf[:], start=True, stop=True)
            tmph = p_w.tile([EMB, TN], F32, tag="tmph")
            nc.vector.tensor_tensor(tmph[:], h2s[0:EMB, :], stp[0:EMB, :], op=ALU.mult)
            h_fm = p_w.tile([128, TN], MMDT, tag="h_fm")
            nc.vector.scalar_tensor_tensor(h_fm[0:EMB, :], tmph[:], c["beta"][:],
                                           stp[EMB:128, :], op0=ALU.add, op1=ALU.add)
            nc.vector.tensor_copy(h_fm[EMB:128, :], h_fm[0:EMB, :])

            # ---------- user embedding gather ----------
            u_g = p_w.tile([128, TN // 8], F32, tag="u_g")
            nc.gpsimd.ap_gather(u_g[:], c["ut"][:], ids_t[:], channels=128,
                                num_elems=NUSERS, d=1, num_idxs=TN // 8)
            T1ps = ps_m.tile([64, 128], F32, tag="psm", name=f"T1ps_{it}")
            nc.tensor.transpose(T1ps[:], u_g[:], ident[:])
            T1sb = p_w.tile([64, 128], F32, tag="T1sb")
            nc.vector.tensor_copy(T1sb[:], T1ps[:])
            uTps = ps_m.tile([UDIM, TN], F32, tag="psm", name=f"uTps_{it}")
            for g in range(8):
                nc.tensor.transpose(uTps[:, 64 * g:64 * (g + 1)],
                                    T1sb[:, 16 * g:16 * (g + 1)],
                                    ident[0:64, 0:64])
            u_fm = p_w.tile([UDIM, TN], MMDT, tag="u_fm")
            nc.vector.tensor_copy(u_fm[:], uTps[:])

            # ---------- gate ----------
            psU = ps_m.tile([128, TN], F32, tag="psm")
            nc.tensor.matmul(psU[:], c["wgU"][:], h_fm[0:EMB, :], start=True, stop=True)
            psV = ps_m.tile([128, TN], F32, tag="psm")
            nc.tensor.matmul(psV[:], c["wgV"][:], u_fm[:], start=True, stop=True)
            uVs = p_w.tile([128, TN], F32, tag="uVs")
            nc.vector.tensor_copy(uVs[:], psV[:])
            gprod = p_w.tile([128, TN], MMDT, tag="gprod")
            nc.vector.tensor_tensor(gprod[:], psU[:], uVs[:], op=ALU.mult)
            psg = ps_m.tile([E, TN], F32, tag="psm")
            nc.tensor.matmul(psg[:], c["gsum"][:], gprod[:], start=True, stop=True)

            # ---------- experts fc1 (+gelu), z^2 ----------
            z_sb = []
            for q in range(4):
                zqA = ps_z.tile([128, TN], F32, tag="zps", name=f"zqA_{it}_{q}")
                zqB = ps_z.tile([128, TN], F32, tag="zps", name=f"zqB_{it}_{q}")
                nc.tensor.matmul(zqA[:], c["we1"][0:EMB, q, :], h_fm[0:EMB, :],
                                 start=True, stop=True, tile_position=(0, 0))
                nc.tensor.matmul(zqB[:], c["we1"][EMB:128, q, :],
                                 h_fm[EMB:128, :], start=True, stop=True,
                                 tile_position=(EMB, 0))
                for s, zq in enumerate((zqA, zqB)):
                    p = 2 * q + s
                    z = p_z.tile([128, TN], MMDT, tag="z_sb", name=f"z_{it}_{p}")
                    nc.scalar.activation(z[:], zq[:], AF.Gelu,
                                         bias=c["eb1"][:, p:p + 1])
                    z_sb.append(z)

            z2_sb = []
            for p in range(8):
                z2 = p_z2.tile([128, TN], MMDT, tag="z2_sb")
                eng = nc.gpsimd if p < 6 else nc.vector
                eng.tensor_tensor(z2[:], z_sb[p][:], z_sb[p][:], op=ALU.mult)
                z2_sb.append(z2)

            # ---------- expert stats (z^2) and fc2 (+mu), col-tiled ----------
            zst = [ps_m.tile([128, TN], F32, tag="psm", name=f"zst{it}_{i}") for i in range(2)]
            for grp in range(2):
                for j in range(4):
                    p = 4 * grp + j
                    nc.tensor.matmul(zst[grp][32 * j:32 * j + 32, :], c["stat64"][:],
                                     z2_sb[p][:], start=True, stop=True,
                                     tile_position=(0, 32 * j))
            fc2 = [ps_f.tile([128, TN], F32, tag="fc2", name=f"fc2_{it}_{i}") for i in range(2)]
            for grp in range(2):
                for j in range(4):
                    p = 4 * grp + j
                    nc.tensor.matmul(fc2[grp][32 * j:32 * j + 32, :],
                                     c["we2"][:, p, :], z_sb[p][:],
                                     start=True, stop=True, tile_position=(0, 32 * j))

            # ---------- stats to batch-major via full-bank transposes ----------
            # copy fc2 / zst psum banks to SBUF (fc2sb also feeds combine)
            fc2sb, zstsb = [], []
            for b in range(2):
                t = p_w.tile([128, TN], F32, tag="fc2sb", name=f"fc2sb_{it}_{b}")
                nc.vector.tensor_copy(t[:], fc2[b][:])
                fc2sb.append(t)
                t2 = p_w.tile([128, TN], F32, tag="zstsb", name=f"zstsb_{it}_{b}")
                nc.any.tensor_copy(t2[:], zst[b][:])
                zstsb.append(t2)
            g_sb = p_sc.tile([E, TN], F32, tag="g_sb")
            nc.vector.tensor_scalar(g_sb[:], psg[:], c["gb"][:], None, op0=ALU.add)

            muB = p_sc.tile([128, NCH, E], F32, tag="muB")
            m2B = p_sc.tile([128, NCH, E], F32, tag="m2B")

            def _extract(src_ps, dst, base):
                sap = src_ps[:, :, 0]
                a = sap.ap
                sap2 = bass.AP(tensor=sap.tensor, offset=sap.offset + base,
                               ap=[a[0], a[1], [32, 4], [1, 2]])
                dap = dst.ap
                dst2 = bass.AP(tensor=dst.tensor, offset=dst.offset,
                               ap=[dap[0], dap[1], [2, 4], [1, 2]])
                nc.vector.tensor_copy(dst2, sap2)

            for b in range(2):
                psT = ps_m.tile([128, NCH, 128], F32, tag="psm", name=f"psTf_{it}_{b}")
                for ch in range(NCH):
                    nc.tensor.transpose(psT[:, ch, :],
                                        fc2sb[b][:, 128 * ch:128 * (ch + 1)], ident[:])
                _extract(psT, muB[:, :, 8 * b:8 * b + 8], 20)
            for b in range(2):
                psT = ps_m.tile([128, NCH, 128], F32, tag="psm", name=f"psTz_{it}_{b}")
                for ch in range(NCH):
                    nc.tensor.transpose(psT[:, ch, :],
                                        zstsb[b][:, 128 * ch:128 * (ch + 1)], ident[:])
                _extract(psT, m2B[:, :, 8 * b:8 * b + 8], 0)

            psTg = ps_m.tile([128, NCH, E], F32, tag="psm", name=f"psTg_{it}")
            for ch in range(NCH):
                nc.tensor.transpose(psTg[:, ch, :], g_sb[:, 128 * ch:128 * (ch + 1)],
                                    ident[0:E, 0:E])
            gcp = p_sc.tile([128, NCH, E], F32, tag="gcp")
            nc.vector.tensor_copy(gcp[:], psTg[:])

            # ---------- pass B math ----------
            tmpB = p_sc.tile([128, NCH, E], F32, tag="tmpB")
            nc.vector.tensor_tensor(tmpB[:], muB[:], muB[:], op=ALU.mult)
            vB = p_sc.tile([128, NCH, E], F32, tag="vB")
            nc.vector.scalar_tensor_tensor(vB[:], m2B[:], EPS_LN, tmpB[:],
                                           op0=ALU.add, op1=ALU.subtract)
            rsB = p_sc.tile([128, NCH, E], F32, tag="rsB")
            _newton_rsqrt(nc, p_sc, vB[:], rsB[:], [128, NCH, E], "nB")
            vm8 = p_sc.tile([128, NCH, 8], F32, tag="vm8")
            for ch in range(NCH):
                nc.vector.max(vm8[:, ch, :], gcp[:, ch, :])
            dg = p_sc.tile([128, NCH], F32, tag="dg")
            nc.vector.tensor_tensor(dg[:], vm8[:, :, 0], vm8[:, :, 1], op=ALU.subtract)
            th = p_sc.tile([128, NCH], F32, tag="th")
            nc.scalar.activation(th[:], dg[:], AF.Tanh, scale=0.5)
            w12 = p_sc.tile([128, NCH, 2], F32, tag="w12")
            nc.vector.tensor_scalar(w12[:, :, 0], th[:], 0.5, 0.5, op0=ALU.mult, op1=ALU.add)
            nc.vector.tensor_scalar(w12[:, :, 1], th[:], -0.5, 0.5, op0=ALU.mult, op1=ALU.add)

            is1 = p_sc.tile([128, NCH, E], F32, tag="is1")
            nc.vector.tensor_tensor(is1[:], gcp[:], _bc(vm8[:, :, 0:1], E), op=ALU.is_equal)
            is2 = p_sc.tile([128, NCH, E], F32, tag="is2")
            nc.vector.tensor_tensor(is2[:], gcp[:], _bc(vm8[:, :, 1:2], E), op=ALU.is_equal)
            w1t = p_sc.tile([128, NCH, E], F32, tag="w1t")
            nc.vector.tensor_tensor(w1t[:], is1[:], _bc(w12[:, :, 0:1], E), op=ALU.mult)
            w2t = p_sc.tile([128, NCH, E], F32, tag="w2t")
            nc.vector.tensor_tensor(w2t[:], is2[:], _bc(w12[:, :, 1:2], E), op=ALU.mult)

            # back block: cols 0-15 wsm, 16-31 w, 32-47 ws
            backB = p_sc.tile([128, NCH, 48], F32, tag="backB")
            nc.vector.tensor_tensor(backB[:, :, 16:32], w1t[:], w2t[:], op=ALU.add)
            nc.vector.tensor_tensor(backB[:, :, 32:48], backB[:, :, 16:32], rsB[:], op=ALU.mult)
            nc.vector.tensor_tensor(backB[:, :, 0:16], backB[:, :, 32:48], muB[:],
                                    op=ALU.mult)

            psBB = ps_m.tile([48, TN], F32, tag="psm")
            for ch in range(NCH):
                nc.tensor.transpose(psBB[:, 128 * ch:128 * (ch + 1)],
                                    backB[:, ch, :], ident[:])
            cf = p_sc.tile([48, TN], MMDT, tag="cf")
            nc.vector.tensor_copy(cf[:], psBB[:])

            # ---------- combine ----------
            lg = ps_m.tile([NCLS, TN], F32, tag="psm")
            for b in range(2):
                wsr = ps_m.tile([128, TN], F32, tag="psm")
                nc.tensor.matmul(wsr[:], c["wsb"][32:48, b, :], cf[32:48, :],
                                 start=True, stop=True)
                prod = p_w.tile([128, TN], MMDT, tag="prod", name=f"prod_{it}_{b}")
                nc.vector.tensor_tensor(prod[:], fc2sb[b][:], wsr[:], op=ALU.mult)
                nc.tensor.matmul(lg[:], c["msum"][:], prod[:],
                                 start=(b == 0), stop=False)
            nc.tensor.matmul(lg[:], c["gw2c"][:], cf[0:32, :], start=False, stop=True)

            lsb = p_out.tile([NCLS, TN], F32, tag="lsb")
            nc.vector.tensor_copy(lsb[:], lg[:])
            psL = ps_m.tile([128, NCH * NCLS], F32, tag="psm")
            for ch in range(NCH):
                nc.tensor.transpose(psL[:, NCLS * ch:NCLS * (ch + 1)],
                                    lsb[:, 128 * ch:128 * (ch + 1)],
                                    ident[0:NCLS, 0:NCLS])
            osb = p_out.tile([128, NCH, NCLS], F32, tag="osb")
            nc.vector.tensor_copy(osb[:], psL[:])
            nc.sync.dma_start(d_out.ap()[it].rearrange("c p k -> p c k"), osb[:])

    nc.compile()
    return nc


def _newton_rsqrt(nc, pool, v_ap, out_ap, shape, tag):
    """out = 1/sqrt(v) via quake seed + 3 Newton iterations (DVE only)."""
    r = pool.tile(shape, F32, tag=tag + "_r")
    t = pool.tile(shape, F32, tag=tag + "_t")
    nc.vector.tensor_scalar(r[:].bitcast(I32), v_ap.bitcast(I32), 1, None,
                            op0=ALU.logical_shift_right)
    nc.vector.tensor_scalar(r[:].bitcast(I32), r[:].bitcast(I32), -1, 0x5F3759DF,
                            op0=ALU.mult, op1=ALU.add)
    for i in range(3):
        dst = out_ap if i == 2 else r[:]
        nc.vector.tensor_tensor(t[:], r[:], r[:], op=ALU.mult)
        nc.vector.tensor_tensor(t[:], t[:], v_ap, op=ALU.mult)
        nc.vector.tensor_scalar(t[:], t[:], -0.5, 1.5, op0=ALU.mult, op1=ALU.add)
        nc.vector.tensor_tensor(dst, r[:], t[:], op=ALU.mult)


# ---------------------------------------------------------------------------
# host-side weight prep
# ---------------------------------------------------------------------------
def prep_consts(inp):
    f = np.float32
    gU, gV, gb = inp["gU"].astype(f), inp["gV"].astype(f), inp["gb"].astype(f)
    e_w1, e_b1 = inp["e_w1"].astype(f), inp["e_b1"].astype(f)
    e_g, e_beta = inp["e_g"].astype(f), inp["e_beta"].astype(f)
    e_w2, e_b2 = inp["e_w2"].astype(f), inp["e_b2"].astype(f)
    ut = inp["ut"].astype(f)
    bb_g, bb_beta = inp["bb_g"].astype(f), inp["bb_beta"].astype(f)

    cns = {}
    cns["ident"] = np.eye(128, dtype=f)
    cns["wbb1"] = inp["bb_w1"].astype(f)
    cns["wbb2"] = inp["bb_w2"].astype(f)
    cns["b1c"] = inp["bb_b1"].astype(f).reshape(EMB, 1)
    cns["b2c"] = inp["bb_b2"].astype(f).reshape(EMB, 1)
    cns["betac"] = bb_beta.reshape(EMB, 1)

    st = np.zeros((128, 32), f)
    st[0:64, 0] = 1.0 / 64
    st[64:128, 1] = 1.0 / 64
    cns["stat64"] = st

    stl = np.zeros((2, 128), f)
    stl[0, 0:64] = bb_g
    stl[1, 64:128] = -bb_g
    cns["st_lhs"] = stl

    wgU = np.zeros((EMB, 128), f)
    wgV = np.zeros((UDIM, 128), f)
    for e in range(E):
        wgU[:, e * RANK:(e + 1) * RANK] = gU[e]
        wgV[:, e * RANK:(e + 1) * RANK] = gV[e]
    cns["wgU"] = wgU
    cns["wgV"] = wgV

    gs = np.zeros((128, E), f)
    for i, e in enumerate(PERM):
        gs[e * RANK:(e + 1) * RANK, i] = 1.0
    cns["gsum_lhs"] = gs
    cns["gb_col"] = gb[PERM].reshape(E, 1)

    we1 = np.zeros((128, 4, 128), f)
    eb1 = np.zeros((128, 8), f)
    for q in range(4):
        # row-tile A (partitions 0-63) computes pair 2q, tile B pair 2q+1
        we1[0:64, q, :] = np.concatenate([e_w1[4 * q], e_w1[4 * q + 1]], axis=1)
        we1[64:128, q, :] = np.concatenate([e_w1[4 * q + 2], e_w1[4 * q + 3]], axis=1)
    for p in range(8):
        eb1[0:64, p] = e_b1[2 * p]
        eb1[64:128, p] = e_b1[2 * p + 1]
    cns["we1"] = we1
    cns["eb1"] = eb1

    we2 = np.zeros((128, 8, 32), f)
    for p in range(8):
        e0, e1 = 2 * p, 2 * p + 1
        we2[0:64, p, 0:10] = e_g[e0][:, None] * e_w2[e0]
        we2[64:128, p, 10:20] = e_g[e1][:, None] * e_w2[e1]
        we2[0:64, p, 20] = 1.0 / 64
        we2[64:128, p, 21] = 1.0 / 64
    cns["we2"] = we2

    wsb = np.zeros((48, 2, 128), f)
    for i, e in enumerate(PERM):
        p, q = e // 2, e % 2
        b, j = p // 4, p % 4
        wsb[32 + i, b, 32 * j + 10 * q:32 * j + 10 * q + 10] = 1.0
    cns["wsb_lhs"] = wsb

    ms = np.zeros((128, NCLS), f)
    for j in range(4):
        for q in range(2):
            for cc in range(NCLS):
                ms[32 * j + 10 * q + cc, cc] = 1.0
    cns["msum_lhs"] = ms

    gw2 = np.einsum("ed,edc->ec", e_g, e_w2)
    cst = np.einsum("ed,edc->ec", e_beta, e_w2) + e_b2
    gw2c = np.zeros((2 * E, NCLS), f)
    gw2c[0:E] = -gw2[PERM]
    gw2c[E:2 * E] = cst[PERM]
    cns["gw2c_lhs"] = gw2c

    cns["ut_rep"] = np.tile(ut.T, (8, 1)).astype(f)
    return cns


def shard_inputs(x, user_ids, b_core):
    """Returns per-core lists of (x  [nt,4,128,80] f32, ids [nt,128,4] i16)."""
    ncores = x.shape[0] // b_core
    nt = b_core // TN
    xs = np.ascontiguousarray(
        x.reshape(ncores, nt, NCH, 128, IN_F).astype(np.float32))
    ids = user_ids.reshape(ncores, nt, 8, 4, 16).transpose(0, 1, 2, 4, 3)
    ids = np.ascontiguousarray(ids.reshape(ncores, nt, 128, 4)).astype(np.int16)
    return xs, ids


_CACHE = {}


def _get_program(b_core, mmdt):
    key = (b_core, mmdt)
    if key not in _CACHE:
        _CACHE[key] = build_program(b_core, mmdt)
    return _CACHE[key]


def kernel(**inputs):
    from concourse.bass_utils import run_bass_kernel_spmd
    mmdt = os.environ.get("KMMDT", MMDT_DEFAULT)
    x = np.asarray(inputs["x"], np.float32).reshape(B, IN_F)
    uids = np.asarray(inputs["user_ids"]).astype(np.int64)
    nc = _get_program(B_CORE, mmdt)
    cns = prep_consts({k: np.asarray(v) for k, v in inputs.items()})
    xs, ids = shard_inputs(x, uids, B_CORE)
    in_maps = []
    for k in range(NCORES):
        m = dict(cns)
        m["x"] = xs[k]
        m["ids"] = ids[k]
        in_maps.append(m)
    res = run_bass_kernel_spmd(nc, in_maps, core_ids=list(range(NCORES)))
    out = np.concatenate([r["out"].reshape(B_CORE, NCLS) for r in res.results], axis=0)
    return out.astype(np.float32)


# revision 16
# speedup vs baseline: 1.0361x; 1.0361x over previous
"""Trainium2 Bass kernel for nn_MoEClassifier (moe_routing).

Model (per sample):
  x[16,5] -> flat 80 -> fc1(80->64) gelu -> fc2(64->64) gelu -> LN -> h
  u = user_table[user_id]  (16)
  gate: g_e = sum_r (h @ gU[e])_r * (u @ gV[e])_r + gb_e ; top-2 softmax -> w
  experts (dense): z_e = gelu(h @ e_w1[e] + e_b1[e]); LN(z); lpe = z @ e_w2[e] + e_b2
  logits = sum_e w_e * lpe_e   (10 classes)

Strategy: pure data-parallel across 8 NeuronCores (batch 131072 -> 16384/core).
On-chip layout is feature-major ([feature partitions, batch free]).  Per-sample
scalar math (LN rsqrt, top-2 gate) runs batch-major via PE transposes.
Expert LN is folded algebraically into the expert fc2 / combine stage:
  lpe = rs*( (z*g)@w2 - mu*(g@w2) ) + (beta@w2 + b2)
  logits = sum_e ws_e*A_e - sum_e wsm_e*gw2[e] + sum_e w_e*const[e]
with ws = w*rs, wsm = w*rs*mu.
"""
import sys, os

for _p in ("/opt/trn_rl_repo",):
    if _p not in sys.path:
        sys.path.insert(0, _p)

import numpy as np
from contextlib import ExitStack

import concourse.bass as bass
import concourse.tile as tile
from concourse import bacc, mybir

F32 = mybir.dt.float32
F32R = mybir.dt.float32r
I16 = mybir.dt.int16
I32 = mybir.dt.int32
AF = mybir.ActivationFunctionType
ALU = mybir.AluOpType

# Model dims (hardcoded per problem spec)
B = 131072
NCORES = 8
B_CORE = B // NCORES
IN_F = 80
EMB = 64
UDIM = 16
E = 16
RANK = 8
NCLS = 10
NUSERS = 1000
EPS_LN = 1e-5
TN = 512          # streaming tile width (one PSUM bank of fp32)
NCH = TN // 128   # 128-chunks per tile

# expert row order in the per-sample scalar block (see mu/m2 copy layout)
PERM = list(range(16))  # natural order (stats extraction preserves it)

MMDT_DEFAULT = "f32"   # "f32" (exact, 4 cyc/row) or "f32r" (~2e-4 rel; unreliable on HW here)


def _bc(ap, n):
    """broadcast the (size-1) innermost dim of an AP to n via stride 0"""
    return ap.to_broadcast(list(ap.shape[:-1]) + [n])


def build_program(b_core=B_CORE, mmdt=MMDT_DEFAULT, bufs=None):
    MMDT = F32R if mmdt == "f32r" else F32
    ntiles = b_core // TN
    bu = {"inp": 3, "work": 3, "scal": 3, "zsb": 9, "z2sb": 3, "osb": 3,
          "psm": 4, "psz": 2, "psf": 2}
    if bufs:
        bu.update(bufs)
    nc = bacc.Bacc("TRN2", target_bir_lowering=False, debug=False,
                   num_devices=NCORES)

    # ---------------- DRAM I/O ----------------
    d_x = nc.dram_tensor("x", [ntiles, IN_F, TN], MMDT, kind="ExternalInput")
    d_u = nc.dram_tensor("u", [ntiles, UDIM, TN], MMDT, kind="ExternalInput")
    d_out = nc.dram_tensor("out", [ntiles, NCH, 128, NCLS], F32, kind="ExternalOutput")

    def cin(name, shape, dt=F32):
        return nc.dram_tensor(name, shape, dt, kind="ExternalInput")

    d_ident = cin("ident", [128, 128])
    d_wbb1 = cin("wbb1", [IN_F, EMB], MMDT)
    d_wbb2 = cin("wbb2", [EMB, EMB], MMDT)
    d_b1 = cin("b1c", [EMB, 1])
    d_b2 = cin("b2c", [EMB, 1])
    d_beta = cin("betac", [EMB, 1])
    d_stat64 = cin("stat64", [128, 32])
    d_stl = cin("st_lhs", [2, 128], MMDT)
    d_wgU = cin("wgU", [EMB, 128], MMDT)
    d_wgV = cin("wgV", [UDIM, 128], MMDT)
    d_gsum = cin("gsum_lhs", [128, E], MMDT)
    d_gb = cin("gb_col", [E, 1])
    d_we1 = cin("we1", [128, 4, 128], MMDT)
    d_eb1 = cin("eb1", [128, 8])
    d_we2 = cin("we2", [128, 8, 32])
    d_wsb = cin("wsb_lhs", [48, 2, 128], MMDT)
    d_msum = cin("msum_lhs", [128, NCLS], MMDT)
    d_gw2c = cin("gw2c_lhs", [2 * E, NCLS], MMDT)

    with tile.TileContext(nc) as tc, ExitStack() as ctx:
        cpool = ctx.enter_context(tc.tile_pool(name="consts", bufs=1))
        p_in = ctx.enter_context(tc.tile_pool(name="inp", bufs=bu["inp"]))
        p_w = ctx.enter_context(tc.tile_pool(name="work", bufs=bu["work"]))
        p_sc = ctx.enter_context(tc.tile_pool(name="scal", bufs=bu["scal"]))
        p_z = ctx.enter_context(tc.tile_pool(name="zsb", bufs=bu["zsb"]))
        p_z2 = ctx.enter_context(tc.tile_pool(name="z2sb", bufs=bu["z2sb"]))
        p_out = ctx.enter_context(tc.tile_pool(name="osb", bufs=bu["osb"]))
        if bu.get("one_psum"):
            ps_m = ctx.enter_context(tc.tile_pool(name="psall", bufs=bu["one_psum"], space="PSUM"))
            ps_z = ps_m
            ps_f = ps_m
        else:
            ps_m = ctx.enter_context(tc.tile_pool(name="psm", bufs=bu["psm"], space="PSUM"))
            ps_z = ctx.enter_context(tc.tile_pool(name="psz", bufs=bu["psz"], space="PSUM"))
            ps_f = ctx.enter_context(tc.tile_pool(name="psf", bufs=bu["psf"], space="PSUM"))

        # ---------------- constants to SBUF ----------------
        c = {}
        for name, d, shape, dt in [
            ("ident", d_ident, [128, 128], F32),
            ("wbb1", d_wbb1, [IN_F, EMB], MMDT),
            ("wbb2", d_wbb2, [EMB, EMB], MMDT),
            ("b1", d_b1, [EMB, 1], F32),
            ("b2", d_b2, [EMB, 1], F32),
            ("beta", d_beta, [EMB, 1], F32),
            ("stat64", d_stat64, [128, 32], F32),
            ("stl", d_stl, [2, 128], MMDT),
            ("wgU", d_wgU, [EMB, 128], MMDT),
            ("wgV", d_wgV, [UDIM, 128], MMDT),
            ("gsum", d_gsum, [128, E], MMDT),
            ("gb", d_gb, [E, 1], F32),
            ("we1", d_we1, [128, 4, 128], MMDT),
            ("eb1", d_eb1, [128, 8], F32),
            ("we2", d_we2, [128, 8, 32], F32),
            ("wsb", d_wsb, [48, 2, 128], MMDT),
            ("msum", d_msum, [128, NCLS], MMDT),
            ("gw2c", d_gw2c, [2 * E, NCLS], MMDT),
        ]:
            t = cpool.tile(shape, dt, tag=name)
            nc.sync.dma_start(t[:], d.ap())
            c[name] = t

        ident = c["ident"]

        def tile_body(it):
            # ---------- load x / u (feature-major, host-prepped) ----------
            x_fm = p_in.tile([IN_F, TN], MMDT, tag="x_fm")
            nc.sync.dma_start(x_fm[:], d_x.ap()[it])
            u_fm = p_in.tile([UDIM, TN], MMDT, tag="u_fm")
            nc.sync.dma_start(u_fm[:], d_u.ap()[it])

            # ---------- backbone ----------
            ps1 = ps_m.tile([EMB, TN], F32, tag="psm")
            nc.tensor.matmul(ps1[:], c["wbb1"][:], x_fm[:], start=True, stop=True)
            h1 = p_w.tile([EMB, TN], MMDT, tag="h1")
            nc.scalar.activation(h1[:], ps1[:], AF.Gelu, bias=c["b1"][:])

            ps2 = ps_m.tile([EMB, TN], F32, tag="psm")
            nc.tensor.matmul(ps2[:], c["wbb2"][:], h1[:], start=True, stop=True)
            h2s = p_w.tile([128, TN], F32, tag="h2s")   # rows 0-63 h2, 64-127 h2^2
            nc.scalar.activation(h2s[0:EMB, :], ps2[:], AF.Gelu, bias=c["b2"][:])
            nc.scalar.activation(h2s[EMB:128, :], h2s[0:EMB, :], AF.Square)

            psb = ps_m.tile([2, TN], F32, tag="psm")     # mean(h2), mean(h2^2)
            nc.tensor.matmul(psb[:], c["stat64"][:, 0:2], h2s[:], start=True, stop=True)
            stats_bb = p_sc.tile([2, TN], F32, tag="stats_bb")
            nc.vector.tensor_copy(stats_bb[:], psb[:])

            # ---------- pass A: bb LN scalars (batch-major) ----------
            psA = ps_m.tile([128, NCH, 2], F32, tag="psm")
            for ch in range(NCH):
                nc.tensor.transpose(psA[:, ch, :], stats_bb[:, 128 * ch:128 * (ch + 1)],
                                    ident[0:2, 0:2])
            # var = (m2 + eps) - mu^2 ; rs = rsqrt(var) ; p = mu*rs
            sA = p_sc.tile([128, NCH, 2], F32, tag="sA")
            nc.vector.tensor_copy(sA[:], psA[:])
            tmpA = p_sc.tile([128, NCH], F32, tag="tmpA")
            nc.vector.tensor_tensor(tmpA[:], sA[:, :, 0], sA[:, :, 0], op=ALU.mult)
            vA = p_sc.tile([128, NCH], F32, tag="vA")
            nc.vector.scalar_tensor_tensor(vA[:], sA[:, :, 1], EPS_LN, tmpA[:],
                                           op0=ALU.add, op1=ALU.subtract)
            backA = p_sc.tile([128, NCH, 2], F32, tag="backA")
            rsA = backA[:, :, 0]
            _newton_rsqrt(nc, p_sc, vA[:], rsA, [128, NCH], "nA")
            nc.vector.tensor_tensor(backA[:, :, 1], rsA, sA[:, :, 0], op=ALU.mult)

            psBA = ps_m.tile([2, TN], F32, tag="psm")
            for ch in range(NCH):
                nc.tensor.transpose(psBA[:, 128 * ch:128 * (ch + 1)],
                                    backA[:, ch, :], ident[:])
            stf = p_sc.tile([2, TN], MMDT, tag="stf")
            nc.vector.tensor_copy(stf[:], psBA[:])

            # ---------- h = h2*S + (beta + T') ----------
            stp = ps_m.tile([128, TN], F32, tag="psm")
            nc.tensor.matmul(stp[:], c["stl"][:], stf[:], start=True, stop=True)
            tmph = p_w.tile([EMB, TN], F32, tag="tmph")
            nc.vector.tensor_tensor(tmph[:], h2s[0:EMB, :], stp[0:EMB, :], op=ALU.mult)
            h_fm = p_w.tile([128, TN], MMDT, tag="h_fm")
            nc.vector.scalar_tensor_tensor(h_fm[0:EMB, :], tmph[:], c["beta"][:],
                                           stp[EMB:128, :], op0=ALU.add, op1=ALU.add)
            nc.vector.tensor_copy(h_fm[EMB:128, :], h_fm[0:EMB, :])

            # ---------- gate ----------
            psU = ps_m.tile([128, TN], F32, tag="psm")
            nc.tensor.matmul(psU[:], c["wgU"][:], h_fm[0:EMB, :], start=True, stop=True)
            psV = ps_m.tile([128, TN], F32, tag="psm")
            nc.tensor.matmul(psV[:], c["wgV"][:], u_fm[:], start=True, stop=True)
            uVs = p_w.tile([128, TN], F32, tag="uVs")
            nc.scalar.copy(uVs[:], psV[:])
            gprod = p_w.tile([128, TN], MMDT, tag="gprod")
            nc.vector.tensor_tensor(gprod[:], psU[:], uVs[:], op=ALU.mult)
            psg = ps_m.tile([E, TN], F32, tag="psm")
            nc.tensor.matmul(psg[:], c["gsum"][:], gprod[:], start=True, stop=True)

            # ---------- experts fc1 (+gelu), z^2 ----------
            z_sb = []
            for q in range(4):
                zqA = ps_z.tile([128, TN], F32, tag=("psm" if bu.get("one_psum") else "zps"), name=f"zqA_{it}_{q}")
                zqB = ps_z.tile([128, TN], F32, tag=("psm" if bu.get("one_psum") else "zps"), name=f"zqB_{it}_{q}")
                nc.tensor.matmul(zqA[:], c["we1"][0:EMB, q, :], h_fm[0:EMB, :],
                                 start=True, stop=True, tile_position=(0, 0))
                nc.tensor.matmul(zqB[:], c["we1"][EMB:128, q, :],
                                 h_fm[EMB:128, :], start=True, stop=True,
                                 tile_position=(EMB, 0))
                for s, zq in enumerate((zqA, zqB)):
                    p = 2 * q + s
                    z = p_z.tile([128, TN], F32, tag="z_sb", name=f"z_{it}_{p}")
                    nc.scalar.activation(z[:], zq[:], AF.Gelu,
                                         bias=c["eb1"][:, p:p + 1])
                    z_sb.append(z)

            z2_sb = []
            for p in range(8):
                z2 = p_z2.tile([128, TN], F32, tag="z2_sb")
                eng = nc.gpsimd if p < 6 else nc.vector
                eng.tensor_tensor(z2[:], z_sb[p][:], z_sb[p][:], op=ALU.mult)
                z2_sb.append(z2)

            # ---------- expert stats (z^2) and fc2 (+mu), col-tiled ----------
            zst = [ps_m.tile([128, TN], F32, tag="psm", name=f"zst{it}_{i}") for i in range(2)]
            for grp in range(2):
                for j in range(4):
                    p = 4 * grp + j
                    nc.tensor.matmul(zst[grp][32 * j:32 * j + 32, :], c["stat64"][:],
                                     z2_sb[p][:], start=True, stop=True,
                                     tile_position=(0, 32 * j))
            fc2 = [ps_f.tile([128, TN], F32, tag=("psm" if bu.get("one_psum") else "fc2"), name=f"fc2_{it}_{i}") for i in range(2)]
            for grp in range(2):
                for j in range(4):
                    p = 4 * grp + j
                    nc.tensor.matmul(fc2[grp][32 * j:32 * j + 32, :],
                                     c["we2"][:, p, :], z_sb[p][:],
                                     start=True, stop=True, tile_position=(0, 32 * j))

            # ---------- stats to batch-major via full-bank transposes ----------
            # copy fc2 / zst psum banks to SBUF (fc2sb also feeds combine)
            fc2sb, zstsb = [], []
            for b in range(2):
                t = p_w.tile([128, TN], F32, tag="fc2sb", name=f"fc2sb_{it}_{b}")
                nc.vector.tensor_copy(t[:], fc2[b][:])
                fc2sb.append(t)
                t2 = p_w.tile([128, TN], F32, tag="zstsb", name=f"zstsb_{it}_{b}")
                nc.scalar.copy(t2[:], zst[b][:])
                zstsb.append(t2)
            g_sb = p_sc.tile([E, TN], F32, tag="g_sb")
            nc.vector.tensor_scalar(g_sb[:], psg[:], c["gb"][:], None, op0=ALU.add)

            yield  # ---- frontend/backend pipeline split ----

            muB = p_sc.tile([128, NCH, E], F32, tag="muB")
            m2B = p_sc.tile([128, NCH, E], F32, tag="m2B")

            def _extract(src_ps, dst, base):
                sap = src_ps[:, :, 0]
                a = sap.ap
                sap2 = bass.AP(tensor=sap.tensor, offset=sap.offset + base,
                               ap=[a[0], a[1], [32, 4], [1, 2]])
                dap = dst.ap
                dst2 = bass.AP(tensor=dst.tensor, offset=dst.offset,
                               ap=[dap[0], dap[1], [2, 4], [1, 2]])
                nc.vector.tensor_copy(dst2, sap2)

            for b in range(2):
                psT = ps_m.tile([128, NCH, 128], F32, tag="psm", name=f"psTf_{it}_{b}")
                for ch in range(NCH):
                    nc.tensor.transpose(psT[:, ch, :],
                                        fc2sb[b][:, 128 * ch:128 * (ch + 1)], ident[:])
                _extract(psT, muB[:, :, 8 * b:8 * b + 8], 20)
            for b in range(2):
                psT = ps_m.tile([128, NCH, 128], F32, tag="psm", name=f"psTz_{it}_{b}")
                for ch in range(NCH):
                    nc.tensor.transpose(psT[:, ch, :],
                                        zstsb[b][:, 128 * ch:128 * (ch + 1)], ident[:])
                _extract(psT, m2B[:, :, 8 * b:8 * b + 8], 0)

            psTg = ps_m.tile([128, NCH, E], F32, tag="psm", name=f"psTg_{it}")
            for ch in range(NCH):
                nc.tensor.transpose(psTg[:, ch, :], g_sb[:, 128 * ch:128 * (ch + 1)],
                                    ident[0:E, 0:E])
            gcp = p_sc.tile([128, NCH, E], F32, tag="gcp")
            nc.vector.tensor_copy(gcp[:], psTg[:])

            # ---------- pass B math ----------
            tmpB = p_sc.tile([128, NCH, E], F32, tag="tmpB")
            nc.vector.tensor_tensor(tmpB[:], muB[:], muB[:], op=ALU.mult)
            vB = p_sc.tile([128, NCH, E], F32, tag="vB")
            nc.vector.scalar_tensor_tensor(vB[:], m2B[:], EPS_LN, tmpB[:],
                                           op0=ALU.add, op1=ALU.subtract)
            rsB = p_sc.tile([128, NCH, E], F32, tag="rsB")
            _newton_rsqrt(nc, p_sc, vB[:], rsB[:], [128, NCH, E], "nB")
            vm8 = p_sc.tile([128, NCH, 8], F32, tag="vm8")
            for ch in range(NCH):
                nc.vector.max(vm8[:, ch, :], gcp[:, ch, :])
            dg = p_sc.tile([128, NCH], F32, tag="dg")
            nc.vector.tensor_tensor(dg[:], vm8[:, :, 0], vm8[:, :, 1], op=ALU.subtract)
            th = p_sc.tile([128, NCH], F32, tag="th")
            nc.scalar.activation(th[:], dg[:], AF.Tanh, scale=0.5)
            w12 = p_sc.tile([128, NCH, 2], F32, tag="w12")
            nc.vector.tensor_scalar(w12[:, :, 0], th[:], 0.5, 0.5, op0=ALU.mult, op1=ALU.add)
            nc.vector.tensor_scalar(w12[:, :, 1], th[:], -0.5, 0.5, op0=ALU.mult, op1=ALU.add)

            is1 = p_sc.tile([128, NCH, E], F32, tag="is1")
            nc.vector.tensor_tensor(is1[:], gcp[:], _bc(vm8[:, :, 0:1], E), op=ALU.is_equal)
            is2 = p_sc.tile([128, NCH, E], F32, tag="is2")
            nc.vector.tensor_tensor(is2[:], gcp[:], _bc(vm8[:, :, 1:2], E), op=ALU.is_equal)
            w1t = p_sc.tile([128, NCH, E], F32, tag="w1t")
            nc.vector.tensor_tensor(w1t[:], is1[:], _bc(w12[:, :, 0:1], E), op=ALU.mult)
            w2t = p_sc.tile([128, NCH, E], F32, tag="w2t")
            nc.vector.tensor_tensor(w2t[:], is2[:], _bc(w12[:, :, 1:2], E), op=ALU.mult)

            # back block: cols 0-15 wsm, 16-31 w, 32-47 ws
            backB = p_sc.tile([128, NCH, 48], F32, tag="backB")
            nc.vector.tensor_tensor(backB[:, :, 16:32], w1t[:], w2t[:], op=ALU.add)
            nc.vector.tensor_tensor(backB[:, :, 32:48], backB[:, :, 16:32], rsB[:], op=ALU.mult)
            nc.vector.tensor_tensor(backB[:, :, 0:16], backB[:, :, 32:48], muB[:],
                                    op=ALU.mult)

            psBB = ps_m.tile([48, TN], F32, tag="psm")
            for ch in range(NCH):
                nc.tensor.transpose(psBB[:, 128 * ch:128 * (ch + 1)],
                                    backB[:, ch, :], ident[:])
            cf = p_sc.tile([48, TN], MMDT, tag="cf")
            nc.vector.tensor_copy(cf[:], psBB[:])

            # ---------- combine ----------
            lg = ps_m.tile([NCLS, TN], F32, tag="psm")
            for b in range(2):
                wsr = ps_m.tile([128, TN], F32, tag="psm")
                nc.tensor.matmul(wsr[:], c["wsb"][32:48, b, :], cf[32:48, :],
                                 start=True, stop=True)
                prod = p_w.tile([128, TN], MMDT, tag="prod", name=f"prod_{it}_{b}")
                nc.vector.tensor_tensor(prod[:], fc2sb[b][:], wsr[:], op=ALU.mult)
                nc.tensor.matmul(lg[:], c["msum"][:], prod[:],
                                 start=(b == 0), stop=False)
            nc.tensor.matmul(lg[:], c["gw2c"][:], cf[0:32, :], start=False, stop=True)

            lsb = p_out.tile([NCLS, TN], F32, tag="lsb")
            nc.vector.tensor_copy(lsb[:], lg[:])
            psL = ps_m.tile([128, NCH * NCLS], F32, tag="psm")
            for ch in range(NCH):
                nc.tensor.transpose(psL[:, NCLS * ch:NCLS * (ch + 1)],
                                    lsb[:, 128 * ch:128 * (ch + 1)],
                                    ident[0:NCLS, 0:NCLS])
            osb = p_out.tile([128, NCH, NCLS], F32, tag="osb")
            nc.vector.tensor_copy(osb[:], psL[:])
            nc.sync.dma_start(d_out.ap()[it].rearrange("c p k -> p c k"), osb[:])

        gens = []
        for it in range(ntiles):
            gen = tile_body(it)
            next(gen)
            gens.append(gen)
            if it > 0:
                for _ in gens[it - 1]:
                    pass
        for _ in gens[ntiles - 1]:
            pass

    nc.compile()
    return nc


def _newton_rsqrt(nc, pool, v_ap, out_ap, shape, tag):
    """out = 1/sqrt(v) via quake seed + 3 Newton iterations (DVE only)."""
    r = pool.tile(shape, F32, tag=tag + "_r")
    t = pool.tile(shape, F32, tag=tag + "_t")
    nc.vector.tensor_scalar(r[:].bitcast(I32), v_ap.bitcast(I32), 1, None,
                            op0=ALU.logical_shift_right)
    nc.vector.tensor_scalar(r[:].bitcast(I32), r[:].bitcast(I32), -1, 0x5F3759DF,
                            op0=ALU.mult, op1=ALU.add)
    for i in range(3):
        dst = out_ap if i == 2 else r[:]
        nc.vector.tensor_tensor(t[:], r[:], r[:], op=ALU.mult)
        nc.vector.tensor_tensor(t[:], t[:], v_ap, op=ALU.mult)
        nc.vector.tensor_scalar(t[:], t[:], -0.5, 1.5, op0=ALU.mult, op1=ALU.add)
        nc.vector.tensor_tensor(dst, r[:], t[:], op=ALU.mult)


# ---------------------------------------------------------------------------
# host-side weight prep
# ---------------------------------------------------------------------------
def prep_consts(inp):
    f = np.float32
    gU, gV, gb = inp["gU"].astype(f), inp["gV"].astype(f), inp["gb"].astype(f)
    e_w1, e_b1 = inp["e_w1"].astype(f), inp["e_b1"].astype(f)
    e_g, e_beta = inp["e_g"].astype(f), inp["e_beta"].astype(f)
    e_w2, e_b2 = inp["e_w2"].astype(f), inp["e_b2"].astype(f)
    ut = inp["ut"].astype(f)
    bb_g, bb_beta = inp["bb_g"].astype(f), inp["bb_beta"].astype(f)

    cns = {}
    cns["ident"] = np.eye(128, dtype=f)
    cns["wbb1"] = inp["bb_w1"].astype(f)
    cns["wbb2"] = inp["bb_w2"].astype(f)
    cns["b1c"] = inp["bb_b1"].astype(f).reshape(EMB, 1)
    cns["b2c"] = inp["bb_b2"].astype(f).reshape(EMB, 1)
    cns["betac"] = bb_beta.reshape(EMB, 1)

    st = np.zeros((128, 32), f)
    st[0:64, 0] = 1.0 / 64
    st[64:128, 1] = 1.0 / 64
    cns["stat64"] = st

    stl = np.zeros((2, 128), f)
    stl[0, 0:64] = bb_g
    stl[1, 64:128] = -bb_g
    cns["st_lhs"] = stl

    wgU = np.zeros((EMB, 128), f)
    wgV = np.zeros((UDIM, 128), f)
    for e in range(E):
        wgU[:, e * RANK:(e + 1) * RANK] = gU[e]
        wgV[:, e * RANK:(e + 1) * RANK] = gV[e]
    cns["wgU"] = wgU
    cns["wgV"] = wgV

    gs = np.zeros((128, E), f)
    for i, e in enumerate(PERM):
        gs[e * RANK:(e + 1) * RANK, i] = 1.0
    cns["gsum_lhs"] = gs
    cns["gb_col"] = gb[PERM].reshape(E, 1)

    we1 = np.zeros((128, 4, 128), f)
    eb1 = np.zeros((128, 8), f)
    for q in range(4):
        # row-tile A (partitions 0-63) computes pair 2q, tile B pair 2q+1
        we1[0:64, q, :] = np.concatenate([e_w1[4 * q], e_w1[4 * q + 1]], axis=1)
        we1[64:128, q, :] = np.concatenate([e_w1[4 * q + 2], e_w1[4 * q + 3]], axis=1)
    for p in range(8):
        eb1[0:64, p] = e_b1[2 * p]
        eb1[64:128, p] = e_b1[2 * p + 1]
    cns["we1"] = we1
    cns["eb1"] = eb1

    we2 = np.zeros((128, 8, 32), f)
    for p in range(8):
        e0, e1 = 2 * p, 2 * p + 1
        we2[0:64, p, 0:10] = e_g[e0][:, None] * e_w2[e0]
        we2[64:128, p, 10:20] = e_g[e1][:, None] * e_w2[e1]
        we2[0:64, p, 20] = 1.0 / 64
        we2[64:128, p, 21] = 1.0 / 64
    cns["we2"] = we2

    wsb = np.zeros((48, 2, 128), f)
    for i, e in enumerate(PERM):
        p, q = e // 2, e % 2
        b, j = p // 4, p % 4
        wsb[32 + i, b, 32 * j + 10 * q:32 * j + 10 * q + 10] = 1.0
    cns["wsb_lhs"] = wsb

    ms = np.zeros((128, NCLS), f)
    for j in range(4):
        for q in range(2):
            for cc in range(NCLS):
                ms[32 * j + 10 * q + cc, cc] = 1.0
    cns["msum_lhs"] = ms

    gw2 = np.einsum("ed,edc->ec", e_g, e_w2)
    cst = np.einsum("ed,edc->ec", e_beta, e_w2) + e_b2
    gw2c = np.zeros((2 * E, NCLS), f)
    gw2c[0:E] = -gw2[PERM]
    gw2c[E:2 * E] = cst[PERM]
    cns["gw2c_lhs"] = gw2c

    return cns


def shard_inputs(x, user_ids, ut, b_core):
    """x [B,80] -> per-core [nt,80,512] feature-major; u gathered+transposed."""
    ncores = x.shape[0] // b_core
    nt = b_core // TN
    xs = np.ascontiguousarray(
        x.reshape(ncores, nt, TN, IN_F).transpose(0, 1, 3, 2).astype(np.float32))
    u = ut.astype(np.float32)[user_ids]          # [B, 16]
    us = np.ascontiguousarray(
        u.reshape(ncores, nt, TN, UDIM).transpose(0, 1, 3, 2))
    return xs, us


_CACHE = {}


def _get_program(b_core, mmdt):
    key = (b_core, mmdt)
    if key not in _CACHE:
        _CACHE[key] = build_program(b_core, mmdt)
    return _CACHE[key]


def kernel(**inputs):
    from concourse.bass_utils import run_bass_kernel_spmd
    mmdt = os.environ.get("KMMDT", MMDT_DEFAULT)
    x = np.asarray(inputs["x"], np.float32).reshape(B, IN_F)
    uids = np.asarray(inputs["user_ids"]).astype(np.int64)
    nc = _get_program(B_CORE, mmdt)
    cns = prep_consts({k: np.asarray(v) for k, v in inputs.items()})
    xs, us = shard_inputs(x, uids, np.asarray(inputs["ut"]), B_CORE)
    in_maps = []
    for k in range(NCORES):
        m = dict(cns)
        m["x"] = xs[k]
        m["u"] = us[k]
        in_maps.append(m)
    res = run_bass_kernel_spmd(nc, in_maps, core_ids=list(range(NCORES)))
    out = np.concatenate([r["out"].reshape(B_CORE, NCLS) for r in res.results], axis=0)
    return out.astype(np.float32)


# revision 20
# speedup vs baseline: 1.0439x; 1.0076x over previous
"""Trainium2 Bass kernel for nn_MoEClassifier (moe_routing).

Model (per sample):
  x[16,5] -> flat 80 -> fc1(80->64) gelu -> fc2(64->64) gelu -> LN -> h
  u = user_table[user_id]  (16)
  gate: g_e = sum_r (h @ gU[e])_r * (u @ gV[e])_r + gb_e ; top-2 softmax -> w
  experts (dense): z_e = gelu(h @ e_w1[e] + e_b1[e]); LN(z); lpe = z @ e_w2[e] + e_b2
  logits = sum_e w_e * lpe_e   (10 classes)

Strategy: pure data-parallel across 8 NeuronCores (batch 131072 -> 16384/core).
On-chip layout is feature-major ([feature partitions, batch free]).  Per-sample
scalar math (LN rsqrt, top-2 gate) runs batch-major via PE transposes.
Expert LN is folded algebraically into the expert fc2 / combine stage:
  lpe = rs*( (z*g)@w2 - mu*(g@w2) ) + (beta@w2 + b2)
  logits = sum_e ws_e*A_e - sum_e wsm_e*gw2[e] + sum_e w_e*const[e]
with ws = w*rs, wsm = w*rs*mu.
"""
import sys, os

for _p in ("/opt/trn_rl_repo",):
    if _p not in sys.path:
        sys.path.insert(0, _p)

import numpy as np
from contextlib import ExitStack

import concourse.bass as bass
import concourse.tile as tile
from concourse import bacc, mybir

F32 = mybir.dt.float32
F32R = mybir.dt.float32r
I16 = mybir.dt.int16
I32 = mybir.dt.int32
AF = mybir.ActivationFunctionType
ALU = mybir.AluOpType

# Model dims (hardcoded per problem spec)
B = 131072
NCORES = 8
B_CORE = B // NCORES
IN_F = 80
EMB = 64
UDIM = 16
E = 16
RANK = 8
NCLS = 10
NUSERS = 1000
EPS_LN = 1e-5
TN = 512          # streaming tile width (one PSUM bank of fp32)
NCH = TN // 128   # 128-chunks per tile

# expert row order in the per-sample scalar block (see mu/m2 copy layout)
PERM = list(range(16))  # natural order (stats extraction preserves it)

MMDT_DEFAULT = "f32"   # "f32" (exact, 4 cyc/row) or "f32r" (~2e-4 rel; unreliable on HW here)


def _bc(ap, n):
    """broadcast the (size-1) innermost dim of an AP to n via stride 0"""
    return ap.to_broadcast(list(ap.shape[:-1]) + [n])


def build_program(b_core=B_CORE, mmdt=MMDT_DEFAULT, bufs=None):
    MMDT = F32R if mmdt == "f32r" else F32
    ntiles = b_core // TN
    bu = {"inp": 4, "work": 4, "scal": 4, "zsb": 9, "z2sb": 3, "osb": 4,
          "psm": 4, "psz": 2, "psf": 2}
    if bufs:
        bu.update(bufs)
    nc = bacc.Bacc("TRN2", target_bir_lowering=False, debug=False,
                   num_devices=NCORES)

    # ---------------- DRAM I/O ----------------
    d_x = nc.dram_tensor("x", [ntiles, IN_F, TN], MMDT, kind="ExternalInput")
    d_u = nc.dram_tensor("u", [ntiles, UDIM, TN], MMDT, kind="ExternalInput")
    d_out = nc.dram_tensor("out", [ntiles, NCH, 128, NCLS], F32, kind="ExternalOutput")

    def cin(name, shape, dt=F32):
        return nc.dram_tensor(name, shape, dt, kind="ExternalInput")

    d_ident = cin("ident", [128, 128])
    d_wbb1 = cin("wbb1", [IN_F, EMB], MMDT)
    d_wbb2 = cin("wbb2", [EMB, EMB], MMDT)
    d_b1 = cin("b1c", [EMB, 1])
    d_b2 = cin("b2c", [EMB, 1])
    d_beta = cin("betac", [EMB, 1])
    d_stat64 = cin("stat64", [128, 32])
    d_stl = cin("st_lhs", [2, 128], MMDT)
    d_wgU = cin("wgU", [EMB, 128], MMDT)
    d_wgV = cin("wgV", [UDIM, 128], MMDT)
    d_gsum = cin("gsum_lhs", [128, E], MMDT)
    d_gb = cin("gb_col", [E, 1])
    d_we1 = cin("we1", [128, 4, 128], MMDT)
    d_eb1 = cin("eb1", [128, 8])
    d_we2 = cin("we2", [128, 8, 32])
    d_wsb = cin("wsb_lhs", [48, 2, 128], MMDT)
    d_msum = cin("msum_lhs", [128, NCLS], MMDT)
    d_gw2c = cin("gw2c_lhs", [2 * E, NCLS], MMDT)

    with tile.TileContext(nc) as tc, ExitStack() as ctx:
        cpool = ctx.enter_context(tc.tile_pool(name="consts", bufs=1))
        p_in = ctx.enter_context(tc.tile_pool(name="inp", bufs=bu["inp"]))
        p_w = ctx.enter_context(tc.tile_pool(name="work", bufs=bu["work"]))
        p_sc = ctx.enter_context(tc.tile_pool(name="scal", bufs=bu["scal"]))
        p_z = ctx.enter_context(tc.tile_pool(name="zsb", bufs=bu["zsb"]))
        p_z2 = ctx.enter_context(tc.tile_pool(name="z2sb", bufs=bu["z2sb"]))
        p_out = ctx.enter_context(tc.tile_pool(name="osb", bufs=bu["osb"]))
        if bu.get("one_psum"):
            ps_m = ctx.enter_context(tc.tile_pool(name="psall", bufs=bu["one_psum"], space="PSUM"))
            ps_z = ps_m
            ps_f = ps_m
        else:
            ps_m = ctx.enter_context(tc.tile_pool(name="psm", bufs=bu["psm"], space="PSUM"))
            ps_z = ctx.enter_context(tc.tile_pool(name="psz", bufs=bu["psz"], space="PSUM"))
            ps_f = ctx.enter_context(tc.tile_pool(name="psf", bufs=bu["psf"], space="PSUM"))

        # ---------------- constants to SBUF ----------------
        c = {}
        for name, d, shape, dt in [
            ("ident", d_ident, [128, 128], F32),
            ("wbb1", d_wbb1, [IN_F, EMB], MMDT),
            ("wbb2", d_wbb2, [EMB, EMB], MMDT),
            ("b1", d_b1, [EMB, 1], F32),
            ("b2", d_b2, [EMB, 1], F32),
            ("beta", d_beta, [EMB, 1], F32),
            ("stat64", d_stat64, [128, 32], F32),
            ("stl", d_stl, [2, 128], MMDT),
            ("wgU", d_wgU, [EMB, 128], MMDT),
            ("wgV", d_wgV, [UDIM, 128], MMDT),
            ("gsum", d_gsum, [128, E], MMDT),
            ("gb", d_gb, [E, 1], F32),
            ("we1", d_we1, [128, 4, 128], MMDT),
            ("eb1", d_eb1, [128, 8], F32),
            ("we2", d_we2, [128, 8, 32], F32),
            ("wsb", d_wsb, [48, 2, 128], MMDT),
            ("msum", d_msum, [128, NCLS], MMDT),
            ("gw2c", d_gw2c, [2 * E, NCLS], MMDT),
        ]:
            t = cpool.tile(shape, dt, tag=name)
            nc.sync.dma_start(t[:], d.ap())
            c[name] = t

        ident = c["ident"]

        def tile_body(it):
            # ---------- load x / u (feature-major, host-prepped) ----------
            x_fm = p_in.tile([IN_F, TN], MMDT, tag="x_fm")
            nc.sync.dma_start(x_fm[:], d_x.ap()[it])
            u_fm = p_in.tile([UDIM, TN], MMDT, tag="u_fm")
            nc.sync.dma_start(u_fm[:], d_u.ap()[it])

            # ---------- backbone ----------
            ps1 = ps_m.tile([EMB, TN], F32, tag="psm")
            nc.tensor.matmul(ps1[:], c["wbb1"][:], x_fm[:], start=True, stop=True)
            h1 = p_w.tile([EMB, TN], MMDT, tag="h1")
            nc.scalar.activation(h1[:], ps1[:], AF.Gelu, bias=c["b1"][:])

            ps2 = ps_m.tile([EMB, TN], F32, tag="psm")
            nc.tensor.matmul(ps2[:], c["wbb2"][:], h1[:], start=True, stop=True)
            h2s = p_w.tile([128, TN], F32, tag="h2s")   # rows 0-63 h2, 64-127 h2^2
            nc.scalar.activation(h2s[0:EMB, :], ps2[:], AF.Gelu, bias=c["b2"][:])
            nc.scalar.activation(h2s[EMB:128, :], h2s[0:EMB, :], AF.Square)

            psb = ps_m.tile([2, TN], F32, tag="psm")     # mean(h2), mean(h2^2)
            nc.tensor.matmul(psb[:], c["stat64"][:, 0:2], h2s[:], start=True, stop=True)
            stats_bb = p_sc.tile([2, TN], F32, tag="stats_bb")
            nc.vector.tensor_copy(stats_bb[:], psb[:])

            # ---------- pass A: bb LN scalars (batch-major) ----------
            psA = ps_m.tile([128, NCH, 2], F32, tag="psm")
            for ch in range(NCH):
                nc.tensor.transpose(psA[:, ch, :], stats_bb[:, 128 * ch:128 * (ch + 1)],
                                    ident[0:2, 0:2])
            # var = (m2 + eps) - mu^2 ; rs = rsqrt(var) ; p = mu*rs
            sA = p_sc.tile([128, NCH, 2], F32, tag="sA")
            nc.vector.tensor_copy(sA[:], psA[:])
            tmpA = p_sc.tile([128, NCH], F32, tag="tmpA")
            nc.vector.tensor_tensor(tmpA[:], sA[:, :, 0], sA[:, :, 0], op=ALU.mult)
            vA = p_sc.tile([128, NCH], F32, tag="vA")
            nc.vector.scalar_tensor_tensor(vA[:], sA[:, :, 1], EPS_LN, tmpA[:],
                                           op0=ALU.add, op1=ALU.subtract)
            backA = p_sc.tile([128, NCH, 2], F32, tag="backA")
            rsA = backA[:, :, 0]
            _newton_rsqrt(nc, p_sc, vA[:], rsA, [128, NCH], "nA")
            nc.vector.tensor_tensor(backA[:, :, 1], rsA, sA[:, :, 0], op=ALU.mult)

            psBA = ps_m.tile([2, TN], F32, tag="psm")
            for ch in range(NCH):
                nc.tensor.transpose(psBA[:, 128 * ch:128 * (ch + 1)],
                                    backA[:, ch, :], ident[:])
            stf = p_sc.tile([2, TN], MMDT, tag="stf")
            nc.vector.tensor_copy(stf[:], psBA[:])

            # ---------- h = h2*S + (beta + T') ----------
            stp = ps_m.tile([128, TN], F32, tag="psm")
            nc.tensor.matmul(stp[:], c["stl"][:], stf[:], start=True, stop=True)
            tmph = p_w.tile([EMB, TN], F32, tag="tmph")
            nc.vector.tensor_tensor(tmph[:], h2s[0:EMB, :], stp[0:EMB, :], op=ALU.mult)
            h_fm = p_w.tile([128, TN], MMDT, tag="h_fm")
            nc.vector.scalar_tensor_tensor(h_fm[0:EMB, :], tmph[:], c["beta"][:],
                                           stp[EMB:128, :], op0=ALU.add, op1=ALU.add)
            nc.vector.tensor_copy(h_fm[EMB:128, :], h_fm[0:EMB, :])

            # ---------- gate ----------
            psU = ps_m.tile([128, TN], F32, tag="psm")
            nc.tensor.matmul(psU[:], c["wgU"][:], h_fm[0:EMB, :], start=True, stop=True)
            psV = ps_m.tile([128, TN], F32, tag="psm")
            nc.tensor.matmul(psV[:], c["wgV"][:], u_fm[:], start=True, stop=True)
            uVs = p_w.tile([128, TN], F32, tag="uVs")
            nc.scalar.copy(uVs[:], psV[:])
            gprod = p_w.tile([128, TN], MMDT, tag="gprod")
            nc.vector.tensor_tensor(gprod[:], psU[:], uVs[:], op=ALU.mult)
            psg = ps_m.tile([E, TN], F32, tag="psm")
            nc.tensor.matmul(psg[:], c["gsum"][:], gprod[:], start=True, stop=True)

            # ---------- experts fc1 (+gelu), z^2 ----------
            z_sb = []
            for q in range(4):
                zqA = ps_z.tile([128, TN], F32, tag=("psm" if bu.get("one_psum") else "zps"), name=f"zqA_{it}_{q}")
                zqB = ps_z.tile([128, TN], F32, tag=("psm" if bu.get("one_psum") else "zps"), name=f"zqB_{it}_{q}")
                nc.tensor.matmul(zqA[:], c["we1"][0:EMB, q, :], h_fm[0:EMB, :],
                                 start=True, stop=True, tile_position=(0, 0))
                nc.tensor.matmul(zqB[:], c["we1"][EMB:128, q, :],
                                 h_fm[EMB:128, :], start=True, stop=True,
                                 tile_position=(EMB, 0))
                for s, zq in enumerate((zqA, zqB)):
                    p = 2 * q + s
                    z = p_z.tile([128, TN], F32, tag="z_sb", name=f"z_{it}_{p}")
                    nc.scalar.activation(z[:], zq[:], AF.Gelu,
                                         bias=c["eb1"][:, p:p + 1])
                    z_sb.append(z)

            z2_sb = []
            for p in range(8):
                z2 = p_z2.tile([128, TN], F32, tag="z2_sb")
                eng = nc.gpsimd if p < 6 else nc.vector
                eng.tensor_tensor(z2[:], z_sb[p][:], z_sb[p][:], op=ALU.mult)
                z2_sb.append(z2)

            # ---------- expert stats (z^2) and fc2 (+mu), col-tiled ----------
            zst = [ps_m.tile([128, TN], F32, tag="psm", name=f"zst{it}_{i}") for i in range(2)]
            for grp in range(2):
                for j in range(4):
                    p = 4 * grp + j
                    nc.tensor.matmul(zst[grp][32 * j:32 * j + 32, :], c["stat64"][:],
                                     z2_sb[p][:], start=True, stop=True,
                                     tile_position=(0, 32 * j))
            fc2 = [ps_f.tile([128, TN], F32, tag=("psm" if bu.get("one_psum") else "fc2"), name=f"fc2_{it}_{i}") for i in range(2)]
            for grp in range(2):
                for j in range(4):
                    p = 4 * grp + j
                    nc.tensor.matmul(fc2[grp][32 * j:32 * j + 32, :],
                                     c["we2"][:, p, :], z_sb[p][:],
                                     start=True, stop=True, tile_position=(0, 32 * j))

            # ---------- stats to batch-major via full-bank transposes ----------
            # copy fc2 / zst psum banks to SBUF (fc2sb also feeds combine)
            fc2sb, zstsb = [], []
            for b in range(2):
                t = p_w.tile([128, TN], F32, tag="fc2sb", name=f"fc2sb_{it}_{b}")
                nc.scalar.copy(t[:], fc2[b][:])
                fc2sb.append(t)
                t2 = p_w.tile([128, TN], F32, tag="zstsb", name=f"zstsb_{it}_{b}")
                nc.scalar.copy(t2[:], zst[b][:])
                zstsb.append(t2)
            g_sb = p_sc.tile([E, TN], F32, tag="g_sb")
            nc.vector.tensor_scalar(g_sb[:], psg[:], c["gb"][:], None, op0=ALU.add)

            yield  # ---- frontend/backend pipeline split ----

            muB = p_sc.tile([128, NCH, E], F32, tag="muB")
            m2B = p_sc.tile([128, NCH, E], F32, tag="m2B")

            def _extract(src_ps, dst, base):
                sap = src_ps[:, :, 0]
                a = sap.ap
                sap2 = bass.AP(tensor=sap.tensor, offset=sap.offset + base,
                               ap=[a[0], a[1], [32, 4], [1, 2]])
                dap = dst.ap
                dst2 = bass.AP(tensor=dst.tensor, offset=dst.offset,
                               ap=[dap[0], dap[1], [2, 4], [1, 2]])
                nc.vector.tensor_copy(dst2, sap2)

            for b in range(2):
                psT = ps_m.tile([128, NCH, 128], F32, tag="psm", name=f"psTf_{it}_{b}")
                for ch in range(NCH):
                    nc.tensor.transpose(psT[:, ch, :],
                                        fc2sb[b][:, 128 * ch:128 * (ch + 1)], ident[:])
                _extract(psT, muB[:, :, 8 * b:8 * b + 8], 20)
            for b in range(2):
                psT = ps_m.tile([128, NCH, 128], F32, tag="psm", name=f"psTz_{it}_{b}")
                for ch in range(NCH):
                    nc.tensor.transpose(psT[:, ch, :],
                                        zstsb[b][:, 128 * ch:128 * (ch + 1)], ident[:])
                _extract(psT, m2B[:, :, 8 * b:8 * b + 8], 0)

            psTg = ps_m.tile([128, NCH, E], F32, tag="psm", name=f"psTg_{it}")
            for ch in range(NCH):
                nc.tensor.transpose(psTg[:, ch, :], g_sb[:, 128 * ch:128 * (ch + 1)],
                                    ident[0:E, 0:E])
            gcp = p_sc.tile([128, NCH, E], F32, tag="gcp")
            nc.scalar.copy(gcp[:], psTg[:])

            # ---------- pass B math ----------
            tmpB = p_sc.tile([128, NCH, E], F32, tag="tmpB")
            nc.vector.tensor_tensor(tmpB[:], muB[:], muB[:], op=ALU.mult)
            vB = p_sc.tile([128, NCH, E], F32, tag="vB")
            nc.vector.scalar_tensor_tensor(vB[:], m2B[:], EPS_LN, tmpB[:],
                                           op0=ALU.add, op1=ALU.subtract)
            rsB = p_sc.tile([128, NCH, E], F32, tag="rsB")
            _newton_rsqrt(nc, p_sc, vB[:], rsB[:], [128, NCH, E], "nB")
            vm8 = p_sc.tile([128, NCH, 8], F32, tag="vm8")
            for ch in range(NCH):
                nc.vector.max(vm8[:, ch, :], gcp[:, ch, :])
            dg = p_sc.tile([128, NCH], F32, tag="dg")
            nc.vector.tensor_tensor(dg[:], vm8[:, :, 0], vm8[:, :, 1], op=ALU.subtract)
            th = p_sc.tile([128, NCH], F32, tag="th")
            nc.scalar.activation(th[:], dg[:], AF.Tanh, scale=0.5)
            w12 = p_sc.tile([128, NCH, 2], F32, tag="w12")
            nc.vector.tensor_scalar(w12[:, :, 0], th[:], 0.5, 0.5, op0=ALU.mult, op1=ALU.add)
            nc.vector.tensor_scalar(w12[:, :, 1], th[:], -0.5, 0.5, op0=ALU.mult, op1=ALU.add)

            is1 = p_sc.tile([128, NCH, E], F32, tag="is1")
            nc.vector.tensor_tensor(is1[:], gcp[:], _bc(vm8[:, :, 0:1], E), op=ALU.is_equal)
            is2 = p_sc.tile([128, NCH, E], F32, tag="is2")
            nc.vector.tensor_tensor(is2[:], gcp[:], _bc(vm8[:, :, 1:2], E), op=ALU.is_equal)
            w1t = p_sc.tile([128, NCH, E], F32, tag="w1t")
            nc.vector.tensor_tensor(w1t[:], is1[:], _bc(w12[:, :, 0:1], E), op=ALU.mult)
            w2t = p_sc.tile([128, NCH, E], F32, tag="w2t")
            nc.vector.tensor_tensor(w2t[:], is2[:], _bc(w12[:, :, 1:2], E), op=ALU.mult)

            # back block: cols 0-15 wsm, 16-31 w, 32-47 ws, 48-63 pad
            backB = p_sc.tile([128, NCH, 64], F32, tag="backB")
            nc.gpsimd.memset(backB[:].rearrange("p c k -> p (c k)"), 0.0)
            nc.vector.tensor_tensor(backB[:, :, 16:32], w1t[:], w2t[:], op=ALU.add)
            nc.vector.tensor_tensor(backB[:, :, 32:48], backB[:, :, 16:32], rsB[:], op=ALU.mult)
            nc.vector.tensor_tensor(backB[:, :, 0:16], backB[:, :, 32:48], muB[:],
                                    op=ALU.mult)

            # 2 transposes of [128,128] (chunk-pairs, 64-padded); cf de-interleaves
            psBB = ps_m.tile([128, 2, 128], F32, tag="psm")
            backBv = backB[:].rearrange("p c k -> p (c k)")
            for hh in range(2):
                nc.tensor.transpose(psBB[:, hh, :],
                                    backBv[:, 128 * hh:128 * (hh + 1)], ident[:])
            cf = p_sc.tile([48, TN], MMDT, tag="cf")
            cfv = cf[:].rearrange("p (h c q) -> p h c q", h=2, c=2, q=128)
            nc.vector.tensor_copy(cfv[:, :, 0, :], psBB[0:48, :, :])
            nc.vector.tensor_copy(cfv[:, :, 1, :], psBB[64:112, :, :])

            # ---------- combine ----------
            lg = ps_m.tile([NCLS, TN], F32, tag="psm")
            for b in range(2):
                wsr = ps_m.tile([128, TN], F32, tag="psm")
                nc.tensor.matmul(wsr[:], c["wsb"][32:48, b, :], cf[32:48, :],
                                 start=True, stop=True)
                prod = p_w.tile([128, TN], MMDT, tag="prod", name=f"prod_{it}_{b}")
                nc.vector.tensor_tensor(prod[:], fc2sb[b][:], wsr[:], op=ALU.mult)
                nc.tensor.matmul(lg[:], c["msum"][:], prod[:],
                                 start=(b == 0), stop=False)
            nc.tensor.matmul(lg[:], c["gw2c"][:], cf[0:32, :], start=False, stop=True)

            lsb = p_out.tile([NCLS, TN], F32, tag="lsb")
            nc.scalar.copy(lsb[:], lg[:])
            psL = ps_m.tile([128, NCH * NCLS], F32, tag="psm")
            for ch in range(NCH):
                nc.tensor.transpose(psL[:, NCLS * ch:NCLS * (ch + 1)],
                                    lsb[:, 128 * ch:128 * (ch + 1)],
                                    ident[0:NCLS, 0:NCLS])
            osb = p_out.tile([128, NCH, NCLS], F32, tag="osb")
            nc.vector.tensor_copy(osb[:], psL[:])
            nc.sync.dma_start(d_out.ap()[it].rearrange("c p k -> p c k"), osb[:])

        SKEW = int(os.environ.get("KSKEW", "1"))
        gens = []
        for it in range(ntiles):
            gen = tile_body(it)
            next(gen)
            gens.append(gen)
            if it >= SKEW:
                for _ in gens[it - SKEW]:
                    pass
        for it in range(max(0, ntiles - SKEW), ntiles):
            for _ in gens[it]:
                pass

    nc.compile()
    return nc


def _newton_rsqrt(nc, pool, v_ap, out_ap, shape, tag):
    """out = 1/sqrt(v) via quake seed + 3 Newton iterations (DVE only)."""
    r = pool.tile(shape, F32, tag=tag + "_r")
    t = pool.tile(shape, F32, tag=tag + "_t")
    nc.vector.tensor_scalar(r[:].bitcast(I32), v_ap.bitcast(I32), 1, None,
                            op0=ALU.logical_shift_right)
    nc.vector.tensor_scalar(r[:].bitcast(I32), r[:].bitcast(I32), -1, 0x5F3759DF,
                            op0=ALU.mult, op1=ALU.add)
    for i in range(3):
        dst = out_ap if i == 2 else r[:]
        nc.vector.tensor_tensor(t[:], r[:], r[:], op=ALU.mult)
        nc.vector.tensor_tensor(t[:], t[:], v_ap, op=ALU.mult)
        nc.vector.tensor_scalar(t[:], t[:], -0.5, 1.5, op0=ALU.mult, op1=ALU.add)
        nc.vector.tensor_tensor(dst, r[:], t[:], op=ALU.mult)


# ---------------------------------------------------------------------------
# host-side weight prep
# ---------------------------------------------------------------------------
def prep_consts(inp):
    f = np.float32
    gU, gV, gb = inp["gU"].astype(f), inp["gV"].astype(f), inp["gb"].astype(f)
    e_w1, e_b1 = inp["e_w1"].astype(f), inp["e_b1"].astype(f)
    e_g, e_beta = inp["e_g"].astype(f), inp["e_beta"].astype(f)
    e_w2, e_b2 = inp["e_w2"].astype(f), inp["e_b2"].astype(f)
    ut = inp["ut"].astype(f)
    bb_g, bb_beta = inp["bb_g"].astype(f), inp["bb_beta"].astype(f)

    cns = {}
    cns["ident"] = np.eye(128, dtype=f)
    cns["wbb1"] = inp["bb_w1"].astype(f)
    cns["wbb2"] = inp["bb_w2"].astype(f)
    cns["b1c"] = inp["bb_b1"].astype(f).reshape(EMB, 1)
    cns["b2c"] = inp["bb_b2"].astype(f).reshape(EMB, 1)
    cns["betac"] = bb_beta.reshape(EMB, 1)

    st = np.zeros((128, 32), f)
    st[0:64, 0] = 1.0 / 64
    st[64:128, 1] = 1.0 / 64
    cns["stat64"] = st

    stl = np.zeros((2, 128), f)
    stl[0, 0:64] = bb_g
    stl[1, 64:128] = -bb_g
    cns["st_lhs"] = stl

    wgU = np.zeros((EMB, 128), f)
    wgV = np.zeros((UDIM, 128), f)
    for e in range(E):
        wgU[:, e * RANK:(e + 1) * RANK] = gU[e]
        wgV[:, e * RANK:(e + 1) * RANK] = gV[e]
    cns["wgU"] = wgU
    cns["wgV"] = wgV

    gs = np.zeros((128, E), f)
    for i, e in enumerate(PERM):
        gs[e * RANK:(e + 1) * RANK, i] = 1.0
    cns["gsum_lhs"] = gs
    cns["gb_col"] = gb[PERM].reshape(E, 1)

    we1 = np.zeros((128, 4, 128), f)
    eb1 = np.zeros((128, 8), f)
    for q in range(4):
        # row-tile A (partitions 0-63) computes pair 2q, tile B pair 2q+1
        we1[0:64, q, :] = np.concatenate([e_w1[4 * q], e_w1[4 * q + 1]], axis=1)
        we1[64:128, q, :] = np.concatenate([e_w1[4 * q + 2], e_w1[4 * q + 3]], axis=1)
    for p in range(8):
        eb1[0:64, p] = e_b1[2 * p]
        eb1[64:128, p] = e_b1[2 * p + 1]
    cns["we1"] = we1
    cns["eb1"] = eb1

    we2 = np.zeros((128, 8, 32), f)
    for p in range(8):
        e0, e1 = 2 * p, 2 * p + 1
        we2[0:64, p, 0:10] = e_g[e0][:, None] * e_w2[e0]
        we2[64:128, p, 10:20] = e_g[e1][:, None] * e_w2[e1]
        we2[0:64, p, 20] = 1.0 / 64
        we2[64:128, p, 21] = 1.0 / 64
    cns["we2"] = we2

    wsb = np.zeros((48, 2, 128), f)
    for i, e in enumerate(PERM):
        p, q = e // 2, e % 2
        b, j = p // 4, p % 4
        wsb[32 + i, b, 32 * j + 10 * q:32 * j + 10 * q + 10] = 1.0
    cns["wsb_lhs"] = wsb

    ms = np.zeros((128, NCLS), f)
    for j in range(4):
        for q in range(2):
            for cc in range(NCLS):
                ms[32 * j + 10 * q + cc, cc] = 1.0
    cns["msum_lhs"] = ms

    gw2 = np.einsum("ed,edc->ec", e_g, e_w2)
    cst = np.einsum("ed,edc->ec", e_beta, e_w2) + e_b2
    gw2c = np.zeros((2 * E, NCLS), f)
    gw2c[0:E] = -gw2[PERM]
    gw2c[E:2 * E] = cst[PERM]
    cns["gw2c_lhs"] = gw2c

    return cns


def shard_inputs(x, user_ids, ut, b_core):
    """x [B,80] -> per-core [nt,80,512] feature-major; u gathered+transposed."""
    ncores = x.shape[0] // b_core
    nt = b_core // TN
    xs = np.ascontiguousarray(
        x.reshape(ncores, nt, TN, IN_F).transpose(0, 1, 3, 2).astype(np.float32))
    u = ut.astype(np.float32)[user_ids]          # [B, 16]
    us = np.ascontiguousarray(
        u.reshape(ncores, nt, TN, UDIM).transpose(0, 1, 3, 2))
    return xs, us


_CACHE = {}


def _get_program(b_core, mmdt):
    key = (b_core, mmdt)
    if key not in _CACHE:
        _CACHE[key] = build_program(b_core, mmdt)
    return _CACHE[key]


def kernel(**inputs):
    from concourse.bass_utils import run_bass_kernel_spmd
    mmdt = os.environ.get("KMMDT", MMDT_DEFAULT)
    x = np.asarray(inputs["x"], np.float32).reshape(B, IN_F)
    uids = np.asarray(inputs["user_ids"]).astype(np.int64)
    nc = _get_program(B_CORE, mmdt)
    cns = prep_consts({k: np.asarray(v) for k, v in inputs.items()})
    xs, us = shard_inputs(x, uids, np.asarray(inputs["ut"]), B_CORE)
    in_maps = []
    for k in range(NCORES):
        m = dict(cns)
        m["x"] = xs[k]
        m["u"] = us[k]
        in_maps.append(m)
    res = run_bass_kernel_spmd(nc, in_maps, core_ids=list(range(NCORES)))
    out = np.concatenate([r["out"].reshape(B_CORE, NCLS) for r in res.results], axis=0)
    return out.astype(np.float32)


# revision 21
# speedup vs baseline: 1.3959x; 1.3372x over previous
"""Trainium2 Bass kernel for nn_MoEClassifier (moe_routing).

Model (per sample):
  x[16,5] -> flat 80 -> fc1(80->64) gelu -> fc2(64->64) gelu -> LN -> h
  u = user_table[user_id]  (16)
  gate: g_e = sum_r (h @ gU[e])_r * (u @ gV[e])_r + gb_e ; top-2 softmax -> w
  experts (dense): z_e = gelu(h @ e_w1[e] + e_b1[e]); LN(z); lpe = z @ e_w2[e] + e_b2
  logits = sum_e w_e * lpe_e   (10 classes)

Strategy: pure data-parallel across 8 NeuronCores (batch 131072 -> 16384/core).
On-chip layout is feature-major ([feature partitions, batch free]).  Per-sample
scalar math (LN rsqrt, top-2 gate) runs batch-major via PE transposes.
Expert LN is folded algebraically into the expert fc2 / combine stage:
  lpe = rs*( (z*g)@w2 - mu*(g@w2) ) + (beta@w2 + b2)
  logits = sum_e ws_e*A_e - sum_e wsm_e*gw2[e] + sum_e w_e*const[e]
with ws = w*rs, wsm = w*rs*mu.
"""
import sys, os

for _p in ("/opt/trn_rl_repo",):
    if _p not in sys.path:
        sys.path.insert(0, _p)

import numpy as np
from contextlib import ExitStack

import concourse.bass as bass
import concourse.tile as tile
from concourse import bacc, mybir

F32 = mybir.dt.float32
F32R = mybir.dt.float32r
I16 = mybir.dt.int16
I32 = mybir.dt.int32
AF = mybir.ActivationFunctionType
ALU = mybir.AluOpType

# Model dims (hardcoded per problem spec)
B = 131072
NCORES = 8
B_CORE = B // NCORES
IN_F = 80
EMB = 64
UDIM = 16
E = 16
RANK = 8
NCLS = 10
NUSERS = 1000
EPS_LN = 1e-5
TN = 512          # streaming tile width (one PSUM bank of fp32)
NCH = TN // 128   # 128-chunks per tile

# expert row order in the per-sample scalar block (see mu/m2 copy layout)
PERM = list(range(16))  # natural order (stats extraction preserves it)

MMDT_DEFAULT = "f32"   # "f32" (exact, 4 cyc/row) or "f32r" (~2e-4 rel; unreliable on HW here)


def _bc(ap, n):
    """broadcast the (size-1) innermost dim of an AP to n via stride 0"""
    return ap.to_broadcast(list(ap.shape[:-1]) + [n])


def build_program(b_core=B_CORE, mmdt=MMDT_DEFAULT, bufs=None):
    MMDT = F32R if mmdt == "f32r" else F32
    ntiles = b_core // TN
    bu = {"inp": 4, "work": 4, "scal": 4, "zsb": 9, "z2sb": 3, "osb": 4,
          "psm": 2, "psz": 2, "psf": 2, "psb2": 2}
    if bufs:
        bu.update(bufs)
    nc = bacc.Bacc("TRN2", target_bir_lowering=False, debug=False,
                   num_devices=NCORES)

    # ---------------- DRAM I/O ----------------
    d_x = nc.dram_tensor("x", [ntiles, IN_F, TN], MMDT, kind="ExternalInput")
    d_u = nc.dram_tensor("u", [ntiles, UDIM, TN], MMDT, kind="ExternalInput")
    d_out = nc.dram_tensor("out", [ntiles, NCH, 128, NCLS], F32, kind="ExternalOutput")

    def cin(name, shape, dt=F32):
        return nc.dram_tensor(name, shape, dt, kind="ExternalInput")

    d_ident = cin("ident", [128, 128])
    d_wbb1 = cin("wbb1", [IN_F, EMB], MMDT)
    d_wbb2 = cin("wbb2", [EMB, EMB], MMDT)
    d_b1 = cin("b1c", [EMB, 1])
    d_b2 = cin("b2c", [EMB, 1])
    d_beta = cin("betac", [EMB, 1])
    d_stat64 = cin("stat64", [128, 32])
    d_stl = cin("st_lhs", [2, 128], MMDT)
    d_wgU = cin("wgU", [EMB, 128], MMDT)
    d_wgV = cin("wgV", [UDIM, 128], MMDT)
    d_gsum = cin("gsum_lhs", [128, E], MMDT)
    d_gb = cin("gb_col", [E, 1])
    d_we1 = cin("we1", [128, 4, 128], MMDT)
    d_eb1 = cin("eb1", [128, 8])
    d_we2 = cin("we2", [128, 8, 32])
    d_wsb = cin("wsb_lhs", [48, 2, 128], MMDT)
    d_msum = cin("msum_lhs", [128, NCLS], MMDT)
    d_gw2c = cin("gw2c_lhs", [2 * E, NCLS], MMDT)

    with tile.TileContext(nc) as tc, ExitStack() as ctx:
        cpool = ctx.enter_context(tc.tile_pool(name="consts", bufs=1))
        p_in = ctx.enter_context(tc.tile_pool(name="inp", bufs=bu["inp"]))
        p_w = ctx.enter_context(tc.tile_pool(name="work", bufs=bu["work"]))
        p_sc = ctx.enter_context(tc.tile_pool(name="scal", bufs=bu["scal"]))
        p_z = ctx.enter_context(tc.tile_pool(name="zsb", bufs=bu["zsb"]))
        p_z2 = ctx.enter_context(tc.tile_pool(name="z2sb", bufs=bu["z2sb"]))
        p_out = ctx.enter_context(tc.tile_pool(name="osb", bufs=bu["osb"]))
        ps_m = ctx.enter_context(tc.tile_pool(name="psm", bufs=bu["psm"], space="PSUM"))
        ps_z = ctx.enter_context(tc.tile_pool(name="psz", bufs=bu["psz"], space="PSUM"))
        ps_f = ctx.enter_context(tc.tile_pool(name="psf", bufs=bu["psf"], space="PSUM"))
        ps_b = ctx.enter_context(tc.tile_pool(name="psb2", bufs=bu["psb2"], space="PSUM")) \
            if bu.get("psb2") else ps_m

        # ---------------- constants to SBUF ----------------
        c = {}
        for name, d, shape, dt in [
            ("ident", d_ident, [128, 128], F32),
            ("wbb1", d_wbb1, [IN_F, EMB], MMDT),
            ("wbb2", d_wbb2, [EMB, EMB], MMDT),
            ("b1", d_b1, [EMB, 1], F32),
            ("b2", d_b2, [EMB, 1], F32),
            ("beta", d_beta, [EMB, 1], F32),
            ("stat64", d_stat64, [128, 32], F32),
            ("stl", d_stl, [2, 128], MMDT),
            ("wgU", d_wgU, [EMB, 128], MMDT),
            ("wgV", d_wgV, [UDIM, 128], MMDT),
            ("gsum", d_gsum, [128, E], MMDT),
            ("gb", d_gb, [E, 1], F32),
            ("we1", d_we1, [128, 4, 128], MMDT),
            ("eb1", d_eb1, [128, 8], F32),
            ("we2", d_we2, [128, 8, 32], F32),
            ("wsb", d_wsb, [48, 2, 128], MMDT),
            ("msum", d_msum, [128, NCLS], MMDT),
            ("gw2c", d_gw2c, [2 * E, NCLS], MMDT),
        ]:
            t = cpool.tile(shape, dt, tag=name)
            nc.sync.dma_start(t[:], d.ap())
            c[name] = t

        ident = c["ident"]

        def tile_body(it):
            # ---------- load x / u (feature-major, host-prepped) ----------
            x_fm = p_in.tile([IN_F, TN], MMDT, tag="x_fm")
            nc.sync.dma_start(x_fm[:], d_x.ap()[it])
            u_fm = p_in.tile([UDIM, TN], MMDT, tag="u_fm")
            nc.sync.dma_start(u_fm[:], d_u.ap()[it])

            # ---------- backbone ----------
            ps1 = ps_m.tile([EMB, TN], F32, tag="psm")
            nc.tensor.matmul(ps1[:], c["wbb1"][:], x_fm[:], start=True, stop=True)
            h1 = p_w.tile([EMB, TN], MMDT, tag="h1")
            nc.scalar.activation(h1[:], ps1[:], AF.Gelu, bias=c["b1"][:])

            ps2 = ps_m.tile([EMB, TN], F32, tag="psm")
            nc.tensor.matmul(ps2[:], c["wbb2"][:], h1[:], start=True, stop=True)
            h2s = p_w.tile([128, TN], F32, tag="h2s")   # rows 0-63 h2, 64-127 h2^2
            nc.scalar.activation(h2s[0:EMB, :], ps2[:], AF.Gelu, bias=c["b2"][:])
            nc.scalar.activation(h2s[EMB:128, :], h2s[0:EMB, :], AF.Square)

            psb = ps_m.tile([2, TN], F32, tag="psm")     # mean(h2), mean(h2^2)
            nc.tensor.matmul(psb[:], c["stat64"][:, 0:2], h2s[:], start=True, stop=True)
            stats_bb = p_sc.tile([2, TN], F32, tag="stats_bb")
            nc.vector.tensor_copy(stats_bb[:], psb[:])

            # ---------- pass A: bb LN scalars (batch-major) ----------
            psA = ps_m.tile([128, NCH, 2], F32, tag="psm")
            for ch in range(NCH):
                nc.tensor.transpose(psA[:, ch, :], stats_bb[:, 128 * ch:128 * (ch + 1)],
                                    ident[0:2, 0:2])
            # var = (m2 + eps) - mu^2 ; rs = rsqrt(var) ; p = mu*rs
            sA = p_sc.tile([128, NCH, 2], F32, tag="sA")
            nc.vector.tensor_copy(sA[:], psA[:])
            tmpA = p_sc.tile([128, NCH], F32, tag="tmpA")
            nc.vector.tensor_tensor(tmpA[:], sA[:, :, 0], sA[:, :, 0], op=ALU.mult)
            vA = p_sc.tile([128, NCH], F32, tag="vA")
            nc.vector.scalar_tensor_tensor(vA[:], sA[:, :, 1], EPS_LN, tmpA[:],
                                           op0=ALU.add, op1=ALU.subtract)
            backA = p_sc.tile([128, NCH, 2], F32, tag="backA")
            rsA = backA[:, :, 0]
            _newton_rsqrt(nc, p_sc, vA[:], rsA, [128, NCH], "nA")
            nc.vector.tensor_tensor(backA[:, :, 1], rsA, sA[:, :, 0], op=ALU.mult)

            psBA = ps_m.tile([2, TN], F32, tag="psm")
            for ch in range(NCH):
                nc.tensor.transpose(psBA[:, 128 * ch:128 * (ch + 1)],
                                    backA[:, ch, :], ident[:])
            stf = p_sc.tile([2, TN], MMDT, tag="stf")
            nc.vector.tensor_copy(stf[:], psBA[:])

            # ---------- h = h2*S + (beta + T') ----------
            stp = ps_m.tile([128, TN], F32, tag="psm")
            nc.tensor.matmul(stp[:], c["stl"][:], stf[:], start=True, stop=True)
            tmph = p_w.tile([EMB, TN], F32, tag="tmph")
            nc.vector.tensor_tensor(tmph[:], h2s[0:EMB, :], stp[0:EMB, :], op=ALU.mult)
            h_fm = p_w.tile([128, TN], MMDT, tag="h_fm")
            nc.vector.scalar_tensor_tensor(h_fm[0:EMB, :], tmph[:], c["beta"][:],
                                           stp[EMB:128, :], op0=ALU.add, op1=ALU.add)
            nc.vector.tensor_copy(h_fm[EMB:128, :], h_fm[0:EMB, :])

            # ---------- gate ----------
            psU = ps_m.tile([128, TN], F32, tag="psm")
            nc.tensor.matmul(psU[:], c["wgU"][:], h_fm[0:EMB, :], start=True, stop=True)
            psV = ps_m.tile([128, TN], F32, tag="psm")
            nc.tensor.matmul(psV[:], c["wgV"][:], u_fm[:], start=True, stop=True)
            uVs = p_w.tile([128, TN], F32, tag="uVs")
            nc.scalar.copy(uVs[:], psV[:])
            gprod = p_w.tile([128, TN], MMDT, tag="gprod")
            nc.vector.tensor_tensor(gprod[:], psU[:], uVs[:], op=ALU.mult)
            psg = ps_m.tile([E, TN], F32, tag="psm")
            nc.tensor.matmul(psg[:], c["gsum"][:], gprod[:], start=True, stop=True)

            # ---------- experts fc1 (+gelu), z^2 ----------
            z_sb = []
            for q in range(4):
                zqA = ps_z.tile([128, TN], F32, tag=("psm" if bu.get("one_psum") else "zps"), name=f"zqA_{it}_{q}")
                zqB = ps_z.tile([128, TN], F32, tag=("psm" if bu.get("one_psum") else "zps"), name=f"zqB_{it}_{q}")
                nc.tensor.matmul(zqA[:], c["we1"][0:EMB, q, :], h_fm[0:EMB, :],
                                 start=True, stop=True, tile_position=(0, 0))
                nc.tensor.matmul(zqB[:], c["we1"][EMB:128, q, :],
                                 h_fm[EMB:128, :], start=True, stop=True,
                                 tile_position=(EMB, 0))
                for s, zq in enumerate((zqA, zqB)):
                    p = 2 * q + s
                    z = p_z.tile([128, TN], F32, tag="z_sb", name=f"z_{it}_{p}")
                    nc.scalar.activation(z[:], zq[:], AF.Gelu,
                                         bias=c["eb1"][:, p:p + 1])
                    z_sb.append(z)

            z2_sb = []
            for p in range(8):
                z2 = p_z2.tile([128, TN], F32, tag="z2_sb")
                eng = nc.gpsimd if p < 6 else nc.vector
                eng.tensor_tensor(z2[:], z_sb[p][:], z_sb[p][:], op=ALU.mult)
                z2_sb.append(z2)

            # ---------- expert stats (z^2) and fc2 (+mu), col-tiled ----------
            zst = [ps_m.tile([128, TN], F32, tag="psm", name=f"zst{it}_{i}") for i in range(2)]
            for grp in range(2):
                for j in range(4):
                    p = 4 * grp + j
                    nc.tensor.matmul(zst[grp][32 * j:32 * j + 32, :], c["stat64"][:],
                                     z2_sb[p][:], start=True, stop=True,
                                     tile_position=(0, 32 * j))
            fc2 = [ps_f.tile([128, TN], F32, tag=("psm" if bu.get("one_psum") else "fc2"), name=f"fc2_{it}_{i}") for i in range(2)]
            for grp in range(2):
                for j in range(4):
                    p = 4 * grp + j
                    nc.tensor.matmul(fc2[grp][32 * j:32 * j + 32, :],
                                     c["we2"][:, p, :], z_sb[p][:],
                                     start=True, stop=True, tile_position=(0, 32 * j))

            # ---------- stats to batch-major via full-bank transposes ----------
            # copy fc2 / zst psum banks to SBUF (fc2sb also feeds combine)
            fc2sb, zstsb = [], []
            for b in range(2):
                t = p_w.tile([128, TN], F32, tag="fc2sb", name=f"fc2sb_{it}_{b}")
                nc.scalar.copy(t[:], fc2[b][:])
                fc2sb.append(t)
                t2 = p_w.tile([128, TN], F32, tag="zstsb", name=f"zstsb_{it}_{b}")
                nc.scalar.copy(t2[:], zst[b][:])
                zstsb.append(t2)
            g_sb = p_sc.tile([E, TN], F32, tag="g_sb")
            nc.vector.tensor_scalar(g_sb[:], psg[:], c["gb"][:], None, op0=ALU.add)

            yield  # ---- frontend/backend pipeline split ----

            muB = p_sc.tile([128, NCH, E], F32, tag="muB")
            m2B = p_sc.tile([128, NCH, E], F32, tag="m2B")

            def _extract(src_ps, dst, base):
                sap = src_ps[:, :, 0]
                a = sap.ap
                sap2 = bass.AP(tensor=sap.tensor, offset=sap.offset + base,
                               ap=[a[0], a[1], [32, 4], [1, 2]])
                dap = dst.ap
                dst2 = bass.AP(tensor=dst.tensor, offset=dst.offset,
                               ap=[dap[0], dap[1], [2, 4], [1, 2]])
                nc.vector.tensor_copy(dst2, sap2)

            for b in range(2):
                psT = ps_b.tile([128, NCH, 128], F32, tag="psb2", name=f"psTf_{it}_{b}")
                for ch in range(NCH):
                    nc.tensor.transpose(psT[:, ch, :],
                                        fc2sb[b][:, 128 * ch:128 * (ch + 1)], ident[:])
                _extract(psT, muB[:, :, 8 * b:8 * b + 8], 20)
            for b in range(2):
                psT = ps_b.tile([128, NCH, 128], F32, tag="psb2", name=f"psTz_{it}_{b}")
                for ch in range(NCH):
                    nc.tensor.transpose(psT[:, ch, :],
                                        zstsb[b][:, 128 * ch:128 * (ch + 1)], ident[:])
                _extract(psT, m2B[:, :, 8 * b:8 * b + 8], 0)

            psTg = ps_b.tile([128, NCH, E], F32, tag="psb2", name=f"psTg_{it}")
            for ch in range(NCH):
                nc.tensor.transpose(psTg[:, ch, :], g_sb[:, 128 * ch:128 * (ch + 1)],
                                    ident[0:E, 0:E])
            gcp = p_sc.tile([128, NCH, E], F32, tag="gcp")
            nc.scalar.copy(gcp[:], psTg[:])

            # ---------- pass B math ----------
            tmpB = p_sc.tile([128, NCH, E], F32, tag="tmpB")
            nc.vector.tensor_tensor(tmpB[:], muB[:], muB[:], op=ALU.mult)
            vB = p_sc.tile([128, NCH, E], F32, tag="vB")
            nc.vector.scalar_tensor_tensor(vB[:], m2B[:], EPS_LN, tmpB[:],
                                           op0=ALU.add, op1=ALU.subtract)
            rsB = p_sc.tile([128, NCH, E], F32, tag="rsB")
            _newton_rsqrt(nc, p_sc, vB[:], rsB[:], [128, NCH, E], "nB")
            vm8 = p_sc.tile([128, NCH, 8], F32, tag="vm8")
            for ch in range(NCH):
                nc.vector.max(vm8[:, ch, :], gcp[:, ch, :])
            dg = p_sc.tile([128, NCH], F32, tag="dg")
            nc.vector.tensor_tensor(dg[:], vm8[:, :, 0], vm8[:, :, 1], op=ALU.subtract)
            th = p_sc.tile([128, NCH], F32, tag="th")
            nc.scalar.activation(th[:], dg[:], AF.Tanh, scale=0.5)
            w12 = p_sc.tile([128, NCH, 2], F32, tag="w12")
            nc.vector.tensor_scalar(w12[:, :, 0], th[:], 0.5, 0.5, op0=ALU.mult, op1=ALU.add)
            nc.vector.tensor_scalar(w12[:, :, 1], th[:], -0.5, 0.5, op0=ALU.mult, op1=ALU.add)

            is1 = p_sc.tile([128, NCH, E], F32, tag="is1")
            nc.vector.tensor_tensor(is1[:], gcp[:], _bc(vm8[:, :, 0:1], E), op=ALU.is_equal)
            is2 = p_sc.tile([128, NCH, E], F32, tag="is2")
            nc.vector.tensor_tensor(is2[:], gcp[:], _bc(vm8[:, :, 1:2], E), op=ALU.is_equal)
            w1t = p_sc.tile([128, NCH, E], F32, tag="w1t")
            nc.vector.tensor_tensor(w1t[:], is1[:], _bc(w12[:, :, 0:1], E), op=ALU.mult)
            w2t = p_sc.tile([128, NCH, E], F32, tag="w2t")
            nc.vector.tensor_tensor(w2t[:], is2[:], _bc(w12[:, :, 1:2], E), op=ALU.mult)

            # back block: cols 0-15 wsm, 16-31 w, 32-47 ws, 48-63 pad
            backB = p_sc.tile([128, NCH, 64], F32, tag="backB")
            nc.gpsimd.memset(backB[:].rearrange("p c k -> p (c k)"), 0.0)
            nc.vector.tensor_tensor(backB[:, :, 16:32], w1t[:], w2t[:], op=ALU.add)
            nc.vector.tensor_tensor(backB[:, :, 32:48], backB[:, :, 16:32], rsB[:], op=ALU.mult)
            nc.vector.tensor_tensor(backB[:, :, 0:16], backB[:, :, 32:48], muB[:],
                                    op=ALU.mult)

            # 2 transposes of [128,128] (chunk-pairs, 64-padded); cf de-interleaves
            psBB = ps_b.tile([128, 2, 128], F32, tag="psb2")
            backBv = backB[:].rearrange("p c k -> p (c k)")
            for hh in range(2):
                nc.tensor.transpose(psBB[:, hh, :],
                                    backBv[:, 128 * hh:128 * (hh + 1)], ident[:])
            cf = p_sc.tile([48, TN], MMDT, tag="cf")
            cfv = cf[:].rearrange("p (h c q) -> p h c q", h=2, c=2, q=128)
            nc.vector.tensor_copy(cfv[:, :, 0, :], psBB[0:48, :, :])
            nc.vector.tensor_copy(cfv[:, :, 1, :], psBB[64:112, :, :])

            # ---------- combine ----------
            lg = ps_b.tile([NCLS, TN], F32, tag="psb2")
            for b in range(2):
                wsr = ps_b.tile([128, TN], F32, tag="psb2")
                nc.tensor.matmul(wsr[:], c["wsb"][32:48, b, :], cf[32:48, :],
                                 start=True, stop=True)
                prod = p_w.tile([128, TN], MMDT, tag="prod", name=f"prod_{it}_{b}")
                nc.vector.tensor_tensor(prod[:], fc2sb[b][:], wsr[:], op=ALU.mult)
                nc.tensor.matmul(lg[:], c["msum"][:], prod[:],
                                 start=(b == 0), stop=False)
            nc.tensor.matmul(lg[:], c["gw2c"][:], cf[0:32, :], start=False, stop=True)

            lsb = p_out.tile([NCLS, TN], F32, tag="lsb")
            nc.scalar.copy(lsb[:], lg[:])
            psL = ps_b.tile([128, NCH * NCLS], F32, tag="psb2")
            for ch in range(NCH):
                nc.tensor.transpose(psL[:, NCLS * ch:NCLS * (ch + 1)],
                                    lsb[:, 128 * ch:128 * (ch + 1)],
                                    ident[0:NCLS, 0:NCLS])
            osb = p_out.tile([128, NCH, NCLS], F32, tag="osb")
            nc.vector.tensor_copy(osb[:], psL[:])
            nc.sync.dma_start(d_out.ap()[it].rearrange("c p k -> p c k"), osb[:])

        SKEW = int(os.environ.get("KSKEW", "1"))
        gens = []
        for it in range(ntiles):
            gen = tile_body(it)
            next(gen)
            gens.append(gen)
            if it >= SKEW:
                for _ in gens[it - SKEW]:
                    pass
        for it in range(max(0, ntiles - SKEW), ntiles):
            for _ in gens[it]:
                pass

    nc.compile()
    return nc


def _newton_rsqrt(nc, pool, v_ap, out_ap, shape, tag):
    """out = 1/sqrt(v) via quake seed + 3 Newton iterations (DVE only)."""
    r = pool.tile(shape, F32, tag=tag + "_r")
    t = pool.tile(shape, F32, tag=tag + "_t")
    nc.vector.tensor_scalar(r[:].bitcast(I32), v_ap.bitcast(I32), 1, None,
                            op0=ALU.logical_shift_right)
    nc.vector.tensor_scalar(r[:].bitcast(I32), r[:].bitcast(I32), -1, 0x5F3759DF,
                            op0=ALU.mult, op1=ALU.add)
    niter = int(os.environ.get("KNEWTON", "2"))
    for i in range(niter):
        dst = out_ap if i == niter - 1 else r[:]
        nc.vector.tensor_tensor(t[:], r[:], r[:], op=ALU.mult)
        nc.vector.tensor_tensor(t[:], t[:], v_ap, op=ALU.mult)
        nc.vector.tensor_scalar(t[:], t[:], -0.5, 1.5, op0=ALU.mult, op1=ALU.add)
        nc.vector.tensor_tensor(dst, r[:], t[:], op=ALU.mult)


# ---------------------------------------------------------------------------
# host-side weight prep
# ---------------------------------------------------------------------------
def prep_consts(inp):
    f = np.float32
    gU, gV, gb = inp["gU"].astype(f), inp["gV"].astype(f), inp["gb"].astype(f)
    e_w1, e_b1 = inp["e_w1"].astype(f), inp["e_b1"].astype(f)
    e_g, e_beta = inp["e_g"].astype(f), inp["e_beta"].astype(f)
    e_w2, e_b2 = inp["e_w2"].astype(f), inp["e_b2"].astype(f)
    ut = inp["ut"].astype(f)
    bb_g, bb_beta = inp["bb_g"].astype(f), inp["bb_beta"].astype(f)

    cns = {}
    cns["ident"] = np.eye(128, dtype=f)
    cns["wbb1"] = inp["bb_w1"].astype(f)
    cns["wbb2"] = inp["bb_w2"].astype(f)
    cns["b1c"] = inp["bb_b1"].astype(f).reshape(EMB, 1)
    cns["b2c"] = inp["bb_b2"].astype(f).reshape(EMB, 1)
    cns["betac"] = bb_beta.reshape(EMB, 1)

    st = np.zeros((128, 32), f)
    st[0:64, 0] = 1.0 / 64
    st[64:128, 1] = 1.0 / 64
    cns["stat64"] = st

    stl = np.zeros((2, 128), f)
    stl[0, 0:64] = bb_g
    stl[1, 64:128] = -bb_g
    cns["st_lhs"] = stl

    wgU = np.zeros((EMB, 128), f)
    wgV = np.zeros((UDIM, 128), f)
    for e in range(E):
        wgU[:, e * RANK:(e + 1) * RANK] = gU[e]
        wgV[:, e * RANK:(e + 1) * RANK] = gV[e]
    cns["wgU"] = wgU
    cns["wgV"] = wgV

    gs = np.zeros((128, E), f)
    for i, e in enumerate(PERM):
        gs[e * RANK:(e + 1) * RANK, i] = 1.0
    cns["gsum_lhs"] = gs
    cns["gb_col"] = gb[PERM].reshape(E, 1)

    we1 = np.zeros((128, 4, 128), f)
    eb1 = np.zeros((128, 8), f)
    for q in range(4):
        # row-tile A (partitions 0-63) computes pair 2q, tile B pair 2q+1
        we1[0:64, q, :] = np.concatenate([e_w1[4 * q], e_w1[4 * q + 1]], axis=1)
        we1[64:128, q, :] = np.concatenate([e_w1[4 * q + 2], e_w1[4 * q + 3]], axis=1)
    for p in range(8):
        eb1[0:64, p] = e_b1[2 * p]
        eb1[64:128, p] = e_b1[2 * p + 1]
    cns["we1"] = we1
    cns["eb1"] = eb1

    we2 = np.zeros((128, 8, 32), f)
    for p in range(8):
        e0, e1 = 2 * p, 2 * p + 1
        we2[0:64, p, 0:10] = e_g[e0][:, None] * e_w2[e0]
        we2[64:128, p, 10:20] = e_g[e1][:, None] * e_w2[e1]
        we2[0:64, p, 20] = 1.0 / 64
        we2[64:128, p, 21] = 1.0 / 64
    cns["we2"] = we2

    wsb = np.zeros((48, 2, 128), f)
    for i, e in enumerate(PERM):
        p, q = e // 2, e % 2
        b, j = p // 4, p % 4
        wsb[32 + i, b, 32 * j + 10 * q:32 * j + 10 * q + 10] = 1.0
    cns["wsb_lhs"] = wsb

    ms = np.zeros((128, NCLS), f)
    for j in range(4):
        for q in range(2):
            for cc in range(NCLS):
                ms[32 * j + 10 * q + cc, cc] = 1.0
    cns["msum_lhs"] = ms

    gw2 = np.einsum("ed,edc->ec", e_g, e_w2)
    cst = np.einsum("ed,edc->ec", e_beta, e_w2) + e_b2
    gw2c = np.zeros((2 * E, NCLS), f)
    gw2c[0:E] = -gw2[PERM]
    gw2c[E:2 * E] = cst[PERM]
    cns["gw2c_lhs"] = gw2c

    return cns


def shard_inputs(x, user_ids, ut, b_core):
    """x [B,80] -> per-core [nt,80,512] feature-major; u gathered+transposed."""
    ncores = x.shape[0] // b_core
    nt = b_core // TN
    xs = np.ascontiguousarray(
        x.reshape(ncores, nt, TN, IN_F).transpose(0, 1, 3, 2).astype(np.float32))
    u = ut.astype(np.float32)[user_ids]          # [B, 16]
    us = np.ascontiguousarray(
        u.reshape(ncores, nt, TN, UDIM).transpose(0, 1, 3, 2))
    return xs, us


_CACHE = {}


def _get_program(b_core, mmdt):
    key = (b_core, mmdt)
    if key not in _CACHE:
        _CACHE[key] = build_program(b_core, mmdt)
    return _CACHE[key]


def kernel(**inputs):
    from concourse.bass_utils import run_bass_kernel_spmd
    mmdt = os.environ.get("KMMDT", MMDT_DEFAULT)
    x = np.asarray(inputs["x"], np.float32).reshape(B, IN_F)
    uids = np.asarray(inputs["user_ids"]).astype(np.int64)
    nc = _get_program(B_CORE, mmdt)
    cns = prep_consts({k: np.asarray(v) for k, v in inputs.items()})
    xs, us = shard_inputs(x, uids, np.asarray(inputs["ut"]), B_CORE)
    in_maps = []
    for k in range(NCORES):
        m = dict(cns)
        m["x"] = xs[k]
        m["u"] = us[k]
        in_maps.append(m)
    res = run_bass_kernel_spmd(nc, in_maps, core_ids=list(range(NCORES)))
    out = np.concatenate([r["out"].reshape(B_CORE, NCLS) for r in res.results], axis=0)
    return out.astype(np.float32)


# revision 23
# speedup vs baseline: 1.4061x; 1.0073x over previous
"""Trainium2 Bass kernel for nn_MoEClassifier (moe_routing).

Model (per sample):
  x[16,5] -> flat 80 -> fc1(80->64) gelu -> fc2(64->64) gelu -> LN -> h
  u = user_table[user_id]  (16)
  gate: g_e = sum_r (h @ gU[e])_r * (u @ gV[e])_r + gb_e ; top-2 softmax -> w
  experts (dense): z_e = gelu(h @ e_w1[e] + e_b1[e]); LN(z); lpe = z @ e_w2[e] + e_b2
  logits = sum_e w_e * lpe_e   (10 classes)

Strategy: pure data-parallel across 8 NeuronCores (batch 131072 -> 16384/core).
On-chip layout is feature-major ([feature partitions, batch free]).  Per-sample
scalar math (LN rsqrt, top-2 gate) runs batch-major via PE transposes.
Expert LN is folded algebraically into the expert fc2 / combine stage:
  lpe = rs*( (z*g)@w2 - mu*(g@w2) ) + (beta@w2 + b2)
  logits = sum_e ws_e*A_e - sum_e wsm_e*gw2[e] + sum_e w_e*const[e]
with ws = w*rs, wsm = w*rs*mu.
"""
import sys, os

for _p in ("/opt/trn_rl_repo",):
    if _p not in sys.path:
        sys.path.insert(0, _p)

import numpy as np
from contextlib import ExitStack

import concourse.bass as bass
import concourse.tile as tile
from concourse import bacc, mybir

F32 = mybir.dt.float32
F32R = mybir.dt.float32r
I16 = mybir.dt.int16
I32 = mybir.dt.int32
AF = mybir.ActivationFunctionType
ALU = mybir.AluOpType

# Model dims (hardcoded per problem spec)
B = 131072
NCORES = 8
B_CORE = B // NCORES
IN_F = 80
EMB = 64
UDIM = 16
E = 16
RANK = 8
NCLS = 10
NUSERS = 1000
EPS_LN = 1e-5
TN = 512          # streaming tile width (one PSUM bank of fp32)
NCH = TN // 128   # 128-chunks per tile

# expert row order in the per-sample scalar block (see mu/m2 copy layout)
PERM = list(range(16))  # natural order (stats extraction preserves it)

MMDT_DEFAULT = "f32"   # "f32" (exact, 4 cyc/row) or "f32r" (~2e-4 rel; unreliable on HW here)


def _bc(ap, n):
    """broadcast the (size-1) innermost dim of an AP to n via stride 0"""
    return ap.to_broadcast(list(ap.shape[:-1]) + [n])


def build_program(b_core=B_CORE, mmdt=MMDT_DEFAULT, bufs=None):
    MMDT = F32R if mmdt == "f32r" else F32
    ntiles = b_core // TN
    bu = {"inp": 4, "work": 4, "scal": 4, "zsb": 9, "z2sb": 3, "osb": 4,
          "psm": 2, "psz": 2, "psf": 2, "psb2": 2}
    if bufs:
        bu.update(bufs)
    nc = bacc.Bacc("TRN2", target_bir_lowering=False, debug=False,
                   num_devices=NCORES)

    # ---------------- DRAM I/O ----------------
    d_x = nc.dram_tensor("x", [ntiles, IN_F, TN], MMDT, kind="ExternalInput")
    d_u = nc.dram_tensor("u", [ntiles, UDIM, TN], MMDT, kind="ExternalInput")
    d_out = nc.dram_tensor("out", [ntiles, NCH, 128, NCLS], F32, kind="ExternalOutput")

    def cin(name, shape, dt=F32):
        return nc.dram_tensor(name, shape, dt, kind="ExternalInput")

    d_ident = cin("ident", [128, 128])
    d_wbb1 = cin("wbb1", [IN_F, EMB], MMDT)
    d_wbb2 = cin("wbb2", [EMB, EMB], MMDT)
    d_b1 = cin("b1c", [EMB, 1])
    d_b2 = cin("b2c", [EMB, 1])
    d_beta = cin("betac", [EMB, 1])
    d_stat64 = cin("stat64", [128, 32])
    d_stl = cin("st_lhs", [2, 128], MMDT)
    d_wgU = cin("wgU", [EMB, 128], MMDT)
    d_wgV = cin("wgV", [UDIM, 128], MMDT)
    d_gsum = cin("gsum_lhs", [128, E], MMDT)
    d_gb = cin("gb_col", [E, 1])
    d_we1 = cin("we1", [128, 4, 128], MMDT)
    d_eb1 = cin("eb1", [128, 8])
    d_we2 = cin("we2", [128, 8, 32])
    d_wsb = cin("wsb_lhs", [48, 2, 128], MMDT)
    d_msum = cin("msum_lhs", [128, NCLS], MMDT)
    d_gw2c = cin("gw2c_lhs", [2 * E, NCLS], MMDT)

    with tile.TileContext(nc) as tc, ExitStack() as ctx:
        cpool = ctx.enter_context(tc.tile_pool(name="consts", bufs=1))
        p_in = ctx.enter_context(tc.tile_pool(name="inp", bufs=bu["inp"]))
        p_w = ctx.enter_context(tc.tile_pool(name="work", bufs=bu["work"]))
        p_sc = ctx.enter_context(tc.tile_pool(name="scal", bufs=bu["scal"]))
        p_z = ctx.enter_context(tc.tile_pool(name="zsb", bufs=bu["zsb"]))
        p_z2 = ctx.enter_context(tc.tile_pool(name="z2sb", bufs=bu["z2sb"]))
        p_out = ctx.enter_context(tc.tile_pool(name="osb", bufs=bu["osb"]))
        ps_m = ctx.enter_context(tc.tile_pool(name="psm", bufs=bu["psm"], space="PSUM"))
        ps_z = ctx.enter_context(tc.tile_pool(name="psz", bufs=bu["psz"], space="PSUM"))
        ps_f = ctx.enter_context(tc.tile_pool(name="psf", bufs=bu["psf"], space="PSUM"))
        ps_b = ctx.enter_context(tc.tile_pool(name="psb2", bufs=bu["psb2"], space="PSUM")) \
            if bu.get("psb2") else ps_m

        # ---------------- constants to SBUF ----------------
        c = {}
        for name, d, shape, dt in [
            ("ident", d_ident, [128, 128], F32),
            ("wbb1", d_wbb1, [IN_F, EMB], MMDT),
            ("wbb2", d_wbb2, [EMB, EMB], MMDT),
            ("b1", d_b1, [EMB, 1], F32),
            ("b2", d_b2, [EMB, 1], F32),
            ("beta", d_beta, [EMB, 1], F32),
            ("stat64", d_stat64, [128, 32], F32),
            ("stl", d_stl, [2, 128], MMDT),
            ("wgU", d_wgU, [EMB, 128], MMDT),
            ("wgV", d_wgV, [UDIM, 128], MMDT),
            ("gsum", d_gsum, [128, E], MMDT),
            ("gb", d_gb, [E, 1], F32),
            ("we1", d_we1, [128, 4, 128], MMDT),
            ("eb1", d_eb1, [128, 8], F32),
            ("we2", d_we2, [128, 8, 32], F32),
            ("wsb", d_wsb, [48, 2, 128], MMDT),
            ("msum", d_msum, [128, NCLS], MMDT),
            ("gw2c", d_gw2c, [2 * E, NCLS], MMDT),
        ]:
            t = cpool.tile(shape, dt, tag=name)
            nc.sync.dma_start(t[:], d.ap())
            c[name] = t

        ident = c["ident"]

        def tile_body(it):
            # ---------- load x / u (feature-major, host-prepped) ----------
            x_fm = p_in.tile([IN_F, TN], MMDT, tag="x_fm")
            nc.sync.dma_start(x_fm[:], d_x.ap()[it])
            u_fm = p_in.tile([UDIM, TN], MMDT, tag="u_fm")
            nc.sync.dma_start(u_fm[:], d_u.ap()[it])

            # ---------- backbone ----------
            ps1 = ps_m.tile([EMB, TN], F32, tag="psm")
            nc.tensor.matmul(ps1[:], c["wbb1"][:], x_fm[:], start=True, stop=True)
            h1 = p_w.tile([EMB, TN], MMDT, tag="h1")
            nc.scalar.activation(h1[:], ps1[:], AF.Gelu, bias=c["b1"][:])

            ps2 = ps_m.tile([EMB, TN], F32, tag="psm")
            nc.tensor.matmul(ps2[:], c["wbb2"][:], h1[:], start=True, stop=True)
            h2s = p_w.tile([128, TN], F32, tag="h2s")   # rows 0-63 h2, 64-127 h2^2
            nc.scalar.activation(h2s[0:EMB, :], ps2[:], AF.Gelu, bias=c["b2"][:])
            nc.scalar.activation(h2s[EMB:128, :], h2s[0:EMB, :], AF.Square)

            psb = ps_m.tile([2, TN], F32, tag="psm")     # mean(h2), mean(h2^2)
            nc.tensor.matmul(psb[:], c["stat64"][:, 0:2], h2s[:], start=True, stop=True)
            stats_bb = p_sc.tile([2, TN], F32, tag="stats_bb")
            nc.vector.tensor_copy(stats_bb[:], psb[:])

            # ---------- pass A: bb LN scalars (batch-major) ----------
            psA = ps_m.tile([128, NCH, 2], F32, tag="psm")
            for ch in range(NCH):
                nc.tensor.transpose(psA[:, ch, :], stats_bb[:, 128 * ch:128 * (ch + 1)],
                                    ident[0:2, 0:2])
            # var = (m2 + eps) - mu^2 ; rs = rsqrt(var) ; p = mu*rs
            sA = p_sc.tile([128, NCH, 2], F32, tag="sA")
            nc.vector.tensor_copy(sA[:], psA[:])
            tmpA = p_sc.tile([128, NCH], F32, tag="tmpA")
            nc.vector.tensor_tensor(tmpA[:], sA[:, :, 0], sA[:, :, 0], op=ALU.mult)
            vA = p_sc.tile([128, NCH], F32, tag="vA")
            nc.vector.scalar_tensor_tensor(vA[:], sA[:, :, 1], EPS_LN, tmpA[:],
                                           op0=ALU.add, op1=ALU.subtract)
            backA = p_sc.tile([128, NCH, 2], F32, tag="backA")
            rsA = backA[:, :, 0]
            _newton_rsqrt(nc, p_sc, vA[:], rsA, [128, NCH], "nA")
            nc.vector.tensor_tensor(backA[:, :, 1], rsA, sA[:, :, 0], op=ALU.mult)

            psBA = ps_m.tile([2, TN], F32, tag="psm")
            for ch in range(NCH):
                nc.tensor.transpose(psBA[:, 128 * ch:128 * (ch + 1)],
                                    backA[:, ch, :], ident[:])
            stf = p_sc.tile([2, TN], MMDT, tag="stf")
            nc.vector.tensor_copy(stf[:], psBA[:])

            # ---------- h = h2*S + (beta + T') ----------
            stp = ps_m.tile([128, TN], F32, tag="psm")
            nc.tensor.matmul(stp[:], c["stl"][:], stf[:], start=True, stop=True)
            tmph = p_w.tile([EMB, TN], F32, tag="tmph")
            nc.vector.tensor_tensor(tmph[:], h2s[0:EMB, :], stp[0:EMB, :], op=ALU.mult)
            h_fm = p_w.tile([128, TN], MMDT, tag="h_fm")
            nc.vector.scalar_tensor_tensor(h_fm[0:EMB, :], tmph[:], c["beta"][:],
                                           stp[EMB:128, :], op0=ALU.add, op1=ALU.add)
            nc.vector.tensor_copy(h_fm[EMB:128, :], h_fm[0:EMB, :])

            # ---------- gate ----------
            psU = ps_m.tile([128, TN], F32, tag="psm")
            nc.tensor.matmul(psU[:], c["wgU"][:], h_fm[0:EMB, :], start=True, stop=True)
            psV = ps_m.tile([128, TN], F32, tag="psm")
            nc.tensor.matmul(psV[:], c["wgV"][:], u_fm[:], start=True, stop=True)
            uVs = p_w.tile([128, TN], F32, tag="uVs")
            nc.scalar.copy(uVs[:], psV[:])
            gprod = p_w.tile([128, TN], MMDT, tag="gprod")
            nc.vector.tensor_tensor(gprod[:], psU[:], uVs[:], op=ALU.mult)
            psg = ps_m.tile([E, TN], F32, tag="psm")
            nc.tensor.matmul(psg[:], c["gsum"][:], gprod[:], start=True, stop=True)

            # ---------- experts fc1 (+gelu), z^2 ----------
            z_sb = []
            for q in range(4):
                zqA = ps_z.tile([128, TN], F32, tag=("psm" if bu.get("one_psum") else "zps"), name=f"zqA_{it}_{q}")
                zqB = ps_z.tile([128, TN], F32, tag=("psm" if bu.get("one_psum") else "zps"), name=f"zqB_{it}_{q}")
                nc.tensor.matmul(zqA[:], c["we1"][0:EMB, q, :], h_fm[0:EMB, :],
                                 start=True, stop=True, tile_position=(0, 0))
                nc.tensor.matmul(zqB[:], c["we1"][EMB:128, q, :],
                                 h_fm[EMB:128, :], start=True, stop=True,
                                 tile_position=(EMB, 0))
                for s, zq in enumerate((zqA, zqB)):
                    p = 2 * q + s
                    z = p_z.tile([128, TN], F32, tag="z_sb", name=f"z_{it}_{p}")
                    nc.scalar.activation(z[:], zq[:], AF.Gelu,
                                         bias=c["eb1"][:, p:p + 1])
                    z_sb.append(z)

            z2_sb = []
            for p in range(8):
                z2 = p_z2.tile([128, TN], F32, tag="z2_sb")
                eng = nc.gpsimd if p < 6 else nc.vector
                eng.tensor_tensor(z2[:], z_sb[p][:], z_sb[p][:], op=ALU.mult)
                z2_sb.append(z2)

            # ---------- expert stats (z^2) and fc2 (+mu), col-tiled ----------
            zst = [ps_m.tile([128, TN], F32, tag="psm", name=f"zst{it}_{i}") for i in range(2)]
            for grp in range(2):
                for j in range(4):
                    p = 4 * grp + j
                    nc.tensor.matmul(zst[grp][32 * j:32 * j + 32, :], c["stat64"][:],
                                     z2_sb[p][:], start=True, stop=True,
                                     tile_position=(0, 32 * j))
            fc2 = [ps_f.tile([128, TN], F32, tag=("psm" if bu.get("one_psum") else "fc2"), name=f"fc2_{it}_{i}") for i in range(2)]
            for grp in range(2):
                for j in range(4):
                    p = 4 * grp + j
                    nc.tensor.matmul(fc2[grp][32 * j:32 * j + 32, :],
                                     c["we2"][:, p, :], z_sb[p][:],
                                     start=True, stop=True, tile_position=(0, 32 * j))

            # ---------- stats to batch-major via full-bank transposes ----------
            # copy fc2 / zst psum banks to SBUF (fc2sb also feeds combine)
            fc2sb, zstsb = [], []
            for b in range(2):
                t = p_w.tile([128, TN], F32, tag="fc2sb", name=f"fc2sb_{it}_{b}")
                nc.scalar.copy(t[:], fc2[b][:])
                fc2sb.append(t)
                t2 = p_w.tile([128, TN], F32, tag="zstsb", name=f"zstsb_{it}_{b}")
                nc.scalar.copy(t2[:], zst[b][:])
                zstsb.append(t2)
            g_sb = p_sc.tile([E, TN], F32, tag="g_sb")
            nc.vector.tensor_scalar(g_sb[:], psg[:], c["gb"][:], None, op0=ALU.add)

            yield  # ---- frontend/backend pipeline split ----

            muB = p_sc.tile([128, NCH, E], F32, tag="muB")
            m2B = p_sc.tile([128, NCH, E], F32, tag="m2B")

            def _extract(src_ps, dst, base):
                sap = src_ps[:, :, 0]
                a = sap.ap
                sap2 = bass.AP(tensor=sap.tensor, offset=sap.offset + base,
                               ap=[a[0], a[1], [32, 4], [1, 2]])
                dap = dst.ap
                dst2 = bass.AP(tensor=dst.tensor, offset=dst.offset,
                               ap=[dap[0], dap[1], [2, 4], [1, 2]])
                nc.vector.tensor_copy(dst2, sap2)

            for b in range(2):
                psT = ps_b.tile([128, NCH, 128], F32, tag="psb2", name=f"psTf_{it}_{b}")
                for ch in range(NCH):
                    nc.tensor.transpose(psT[:, ch, :],
                                        fc2sb[b][:, 128 * ch:128 * (ch + 1)], ident[:])
                _extract(psT, muB[:, :, 8 * b:8 * b + 8], 20)
            for b in range(2):
                psT = ps_b.tile([128, NCH, 128], F32, tag="psb2", name=f"psTz_{it}_{b}")
                for ch in range(NCH):
                    nc.tensor.transpose(psT[:, ch, :],
                                        zstsb[b][:, 128 * ch:128 * (ch + 1)], ident[:])
                _extract(psT, m2B[:, :, 8 * b:8 * b + 8], 0)

            psTg = ps_b.tile([128, NCH, E], F32, tag="psb2", name=f"psTg_{it}")
            for ch in range(NCH):
                nc.tensor.transpose(psTg[:, ch, :], g_sb[:, 128 * ch:128 * (ch + 1)],
                                    ident[0:E, 0:E])
            gcp = p_sc.tile([128, NCH, E], F32, tag="gcp")
            nc.scalar.copy(gcp[:], psTg[:])

            # ---------- pass B math ----------
            tmpB = p_sc.tile([128, NCH, E], F32, tag="tmpB")
            nc.vector.tensor_tensor(tmpB[:], muB[:], muB[:], op=ALU.mult)
            vB = p_sc.tile([128, NCH, E], F32, tag="vB")
            nc.vector.scalar_tensor_tensor(vB[:], m2B[:], EPS_LN, tmpB[:],
                                           op0=ALU.add, op1=ALU.subtract)
            rsB = p_sc.tile([128, NCH, E], F32, tag="rsB")
            _newton_rsqrt(nc, p_sc, vB[:], rsB[:], [128, NCH, E], "nB")
            vm8 = p_sc.tile([128, NCH, 8], F32, tag="vm8")
            for ch in range(NCH):
                nc.vector.max(vm8[:, ch, :], gcp[:, ch, :])
            dg = p_sc.tile([128, NCH], F32, tag="dg")
            nc.vector.tensor_tensor(dg[:], vm8[:, :, 0], vm8[:, :, 1], op=ALU.subtract)
            th = p_sc.tile([128, NCH], F32, tag="th")
            nc.scalar.activation(th[:], dg[:], AF.Tanh, scale=0.5)
            w12 = p_sc.tile([128, NCH, 2], F32, tag="w12")
            nc.vector.tensor_scalar(w12[:, :, 0], th[:], 0.5, 0.5, op0=ALU.mult, op1=ALU.add)
            nc.vector.tensor_scalar(w12[:, :, 1], th[:], -0.5, 0.5, op0=ALU.mult, op1=ALU.add)

            is1 = p_sc.tile([128, NCH, E], F32, tag="is1")
            nc.vector.tensor_tensor(is1[:], gcp[:], _bc(vm8[:, :, 0:1], E), op=ALU.is_equal)
            is2 = p_sc.tile([128, NCH, E], F32, tag="is2")
            nc.vector.tensor_tensor(is2[:], gcp[:], _bc(vm8[:, :, 1:2], E), op=ALU.is_equal)
            w1t = p_sc.tile([128, NCH, E], F32, tag="w1t")
            nc.vector.tensor_tensor(w1t[:], is1[:], _bc(w12[:, :, 0:1], E), op=ALU.mult)
            w2t = p_sc.tile([128, NCH, E], F32, tag="w2t")
            nc.vector.tensor_tensor(w2t[:], is2[:], _bc(w12[:, :, 1:2], E), op=ALU.mult)

            # back block: cols 0-15 wsm, 16-31 w, 32-47 ws, 48-63 pad
            backB = p_sc.tile([128, NCH, 64], F32, tag="backB")
            nc.gpsimd.memset(backB[:].rearrange("p c k -> p (c k)"), 0.0)
            nc.vector.tensor_tensor(backB[:, :, 16:32], w1t[:], w2t[:], op=ALU.add)
            nc.vector.tensor_tensor(backB[:, :, 32:48], backB[:, :, 16:32], rsB[:], op=ALU.mult)
            nc.vector.tensor_tensor(backB[:, :, 0:16], backB[:, :, 32:48], muB[:],
                                    op=ALU.mult)

            # 2 transposes of [128,128] (chunk-pairs, 64-padded); cf de-interleaves
            psBB = ps_b.tile([128, 2, 128], F32, tag="psb2")
            backBv = backB[:].rearrange("p c k -> p (c k)")
            for hh in range(2):
                nc.tensor.transpose(psBB[:, hh, :],
                                    backBv[:, 128 * hh:128 * (hh + 1)], ident[:])
            cf = p_sc.tile([48, TN], MMDT, tag="cf")
            cfv = cf[:].rearrange("p (h c q) -> p h c q", h=2, c=2, q=128)
            nc.vector.tensor_copy(cfv[:, :, 0, :], psBB[0:48, :, :])
            nc.vector.tensor_copy(cfv[:, :, 1, :], psBB[64:112, :, :])

            # ---------- combine ----------
            lg = ps_b.tile([NCLS, TN], F32, tag="psb2")
            prods = []
            for b in range(2):
                wsr = ps_b.tile([128, TN], F32, tag="psb2")
                nc.tensor.matmul(wsr[:], c["wsb"][32:48, b, :], cf[32:48, :],
                                 start=True, stop=True)
                prod = p_w.tile([128, TN], F32, tag="prod", name=f"prod_{it}_{b}")
                nc.vector.tensor_tensor(prod[:], fc2sb[b][:], wsr[:], op=ALU.mult)
                prods.append(prod)
            psum2 = p_w.tile([128, TN], MMDT, tag="psum2")
            nc.vector.tensor_tensor(psum2[:], prods[0][:], prods[1][:], op=ALU.add)
            nc.tensor.matmul(lg[:], c["msum"][:], psum2[:], start=True, stop=False)
            nc.tensor.matmul(lg[:], c["gw2c"][:], cf[0:32, :], start=False, stop=True)

            lsb = p_out.tile([NCLS, TN], F32, tag="lsb")
            nc.scalar.copy(lsb[:], lg[:])
            psL = ps_b.tile([128, NCH * NCLS], F32, tag="psb2")
            for ch in range(NCH):
                nc.tensor.transpose(psL[:, NCLS * ch:NCLS * (ch + 1)],
                                    lsb[:, 128 * ch:128 * (ch + 1)],
                                    ident[0:NCLS, 0:NCLS])
            osb = p_out.tile([128, NCH, NCLS], F32, tag="osb")
            nc.vector.tensor_copy(osb[:], psL[:])
            nc.sync.dma_start(d_out.ap()[it].rearrange("c p k -> p c k"), osb[:])

        SKEW = int(os.environ.get("KSKEW", "1"))
        gens = []
        for it in range(ntiles):
            gen = tile_body(it)
            next(gen)
            gens.append(gen)
            if it >= SKEW:
                for _ in gens[it - SKEW]:
                    pass
        for it in range(max(0, ntiles - SKEW), ntiles):
            for _ in gens[it]:
                pass

    nc.compile()
    return nc


def _newton_rsqrt(nc, pool, v_ap, out_ap, shape, tag, eng=None):
    """out = 1/sqrt(v) via quake seed + Newton iterations."""
    eng = eng or nc.vector
    r = pool.tile(shape, F32, tag=tag + "_r")
    t = pool.tile(shape, F32, tag=tag + "_t")
    eng.tensor_scalar(r[:].bitcast(I32), v_ap.bitcast(I32), 1, None,
                      op0=ALU.logical_shift_right)
    eng.tensor_scalar(r[:].bitcast(I32), r[:].bitcast(I32), -1, 0x5F3759DF,
                      op0=ALU.mult, op1=ALU.add)
    niter = int(os.environ.get("KNEWTON", "2"))
    for i in range(niter):
        dst = out_ap if i == niter - 1 else r[:]
        eng.tensor_tensor(t[:], r[:], r[:], op=ALU.mult)
        eng.tensor_tensor(t[:], t[:], v_ap, op=ALU.mult)
        eng.tensor_scalar(t[:], t[:], -0.5, 1.5, op0=ALU.mult, op1=ALU.add)
        eng.tensor_tensor(dst, r[:], t[:], op=ALU.mult)


# ---------------------------------------------------------------------------
# host-side weight prep
# ---------------------------------------------------------------------------
def prep_consts(inp):
    f = np.float32
    gU, gV, gb = inp["gU"].astype(f), inp["gV"].astype(f), inp["gb"].astype(f)
    e_w1, e_b1 = inp["e_w1"].astype(f), inp["e_b1"].astype(f)
    e_g, e_beta = inp["e_g"].astype(f), inp["e_beta"].astype(f)
    e_w2, e_b2 = inp["e_w2"].astype(f), inp["e_b2"].astype(f)
    ut = inp["ut"].astype(f)
    bb_g, bb_beta = inp["bb_g"].astype(f), inp["bb_beta"].astype(f)

    cns = {}
    cns["ident"] = np.eye(128, dtype=f)
    cns["wbb1"] = inp["bb_w1"].astype(f)
    cns["wbb2"] = inp["bb_w2"].astype(f)
    cns["b1c"] = inp["bb_b1"].astype(f).reshape(EMB, 1)
    cns["b2c"] = inp["bb_b2"].astype(f).reshape(EMB, 1)
    cns["betac"] = bb_beta.reshape(EMB, 1)

    st = np.zeros((128, 32), f)
    st[0:64, 0] = 1.0 / 64
    st[64:128, 1] = 1.0 / 64
    cns["stat64"] = st

    stl = np.zeros((2, 128), f)
    stl[0, 0:64] = bb_g
    stl[1, 64:128] = -bb_g
    cns["st_lhs"] = stl

    wgU = np.zeros((EMB, 128), f)
    wgV = np.zeros((UDIM, 128), f)
    for e in range(E):
        wgU[:, e * RANK:(e + 1) * RANK] = gU[e]
        wgV[:, e * RANK:(e + 1) * RANK] = gV[e]
    cns["wgU"] = wgU
    cns["wgV"] = wgV

    gs = np.zeros((128, E), f)
    for i, e in enumerate(PERM):
        gs[e * RANK:(e + 1) * RANK, i] = 1.0
    cns["gsum_lhs"] = gs
    cns["gb_col"] = gb[PERM].reshape(E, 1)

    we1 = np.zeros((128, 4, 128), f)
    eb1 = np.zeros((128, 8), f)
    for q in range(4):
        # row-tile A (partitions 0-63) computes pair 2q, tile B pair 2q+1
        we1[0:64, q, :] = np.concatenate([e_w1[4 * q], e_w1[4 * q + 1]], axis=1)
        we1[64:128, q, :] = np.concatenate([e_w1[4 * q + 2], e_w1[4 * q + 3]], axis=1)
    for p in range(8):
        eb1[0:64, p] = e_b1[2 * p]
        eb1[64:128, p] = e_b1[2 * p + 1]
    cns["we1"] = we1
    cns["eb1"] = eb1

    we2 = np.zeros((128, 8, 32), f)
    for p in range(8):
        e0, e1 = 2 * p, 2 * p + 1
        we2[0:64, p, 0:10] = e_g[e0][:, None] * e_w2[e0]
        we2[64:128, p, 10:20] = e_g[e1][:, None] * e_w2[e1]
        we2[0:64, p, 20] = 1.0 / 64
        we2[64:128, p, 21] = 1.0 / 64
    cns["we2"] = we2

    wsb = np.zeros((48, 2, 128), f)
    for i, e in enumerate(PERM):
        p, q = e // 2, e % 2
        b, j = p // 4, p % 4
        wsb[32 + i, b, 32 * j + 10 * q:32 * j + 10 * q + 10] = 1.0
    cns["wsb_lhs"] = wsb

    ms = np.zeros((128, NCLS), f)
    for j in range(4):
        for q in range(2):
            for cc in range(NCLS):
                ms[32 * j + 10 * q + cc, cc] = 1.0
    cns["msum_lhs"] = ms

    gw2 = np.einsum("ed,edc->ec", e_g, e_w2)
    cst = np.einsum("ed,edc->ec", e_beta, e_w2) + e_b2
    gw2c = np.zeros((2 * E, NCLS), f)
    gw2c[0:E] = -gw2[PERM]
    gw2c[E:2 * E] = cst[PERM]
    cns["gw2c_lhs"] = gw2c

    return cns


def shard_inputs(x, user_ids, ut, b_core):
    """x [B,80] -> per-core [nt,80,512] feature-major; u gathered+transposed."""
    ncores = x.shape[0] // b_core
    nt = b_core // TN
    xs = np.ascontiguousarray(
        x.reshape(ncores, nt, TN, IN_F).transpose(0, 1, 3, 2).astype(np.float32))
    u = ut.astype(np.float32)[user_ids]          # [B, 16]
    us = np.ascontiguousarray(
        u.reshape(ncores, nt, TN, UDIM).transpose(0, 1, 3, 2))
    return xs, us


_CACHE = {}


def _get_program(b_core, mmdt):
    key = (b_core, mmdt)
    if key not in _CACHE:
        _CACHE[key] = build_program(b_core, mmdt)
    return _CACHE[key]


def kernel(**inputs):
    from concourse.bass_utils import run_bass_kernel_spmd
    mmdt = os.environ.get("KMMDT", MMDT_DEFAULT)
    x = np.asarray(inputs["x"], np.float32).reshape(B, IN_F)
    uids = np.asarray(inputs["user_ids"]).astype(np.int64)
    nc = _get_program(B_CORE, mmdt)
    cns = prep_consts({k: np.asarray(v) for k, v in inputs.items()})
    xs, us = shard_inputs(x, uids, np.asarray(inputs["ut"]), B_CORE)
    in_maps = []
    for k in range(NCORES):
        m = dict(cns)
        m["x"] = xs[k]
        m["u"] = us[k]
        in_maps.append(m)
    res = run_bass_kernel_spmd(nc, in_maps, core_ids=list(range(NCORES)))
    out = np.concatenate([r["out"].reshape(B_CORE, NCLS) for r in res.results], axis=0)
    return out.astype(np.float32)


# revision 24
# speedup vs baseline: 1.4164x; 1.0073x over previous
"""Trainium2 Bass kernel for nn_MoEClassifier (moe_routing).

Model (per sample):
  x[16,5] -> flat 80 -> fc1(80->64) gelu -> fc2(64->64) gelu -> LN -> h
  u = user_table[user_id]  (16)
  gate: g_e = sum_r (h @ gU[e])_r * (u @ gV[e])_r + gb_e ; top-2 softmax -> w
  experts (dense): z_e = gelu(h @ e_w1[e] + e_b1[e]); LN(z); lpe = z @ e_w2[e] + e_b2
  logits = sum_e w_e * lpe_e   (10 classes)

Strategy: pure data-parallel across 8 NeuronCores (batch 131072 -> 16384/core).
On-chip layout is feature-major ([feature partitions, batch free]).  Per-sample
scalar math (LN rsqrt, top-2 gate) runs batch-major via PE transposes.
Expert LN is folded algebraically into the expert fc2 / combine stage:
  lpe = rs*( (z*g)@w2 - mu*(g@w2) ) + (beta@w2 + b2)
  logits = sum_e ws_e*A_e - sum_e wsm_e*gw2[e] + sum_e w_e*const[e]
with ws = w*rs, wsm = w*rs*mu.
"""
import sys, os

for _p in ("/opt/trn_rl_repo",):
    if _p not in sys.path:
        sys.path.insert(0, _p)

import numpy as np
from contextlib import ExitStack

import concourse.bass as bass
import concourse.tile as tile
from concourse import bacc, mybir

F32 = mybir.dt.float32
F32R = mybir.dt.float32r
I16 = mybir.dt.int16
I32 = mybir.dt.int32
AF = mybir.ActivationFunctionType
ALU = mybir.AluOpType

# Model dims (hardcoded per problem spec)
B = 131072
NCORES = 8
B_CORE = B // NCORES
IN_F = 80
EMB = 64
UDIM = 16
E = 16
RANK = 8
NCLS = 10
NUSERS = 1000
EPS_LN = 1e-5
TN = 512          # streaming tile width (one PSUM bank of fp32)
NCH = TN // 128   # 128-chunks per tile

# expert row order in the per-sample scalar block (see mu/m2 copy layout)
PERM = list(range(16))  # natural order (stats extraction preserves it)

MMDT_DEFAULT = "f32"   # "f32" (exact, 4 cyc/row) or "f32r" (~2e-4 rel; unreliable on HW here)


def _bc(ap, n):
    """broadcast the (size-1) innermost dim of an AP to n via stride 0"""
    return ap.to_broadcast(list(ap.shape[:-1]) + [n])


def build_program(b_core=B_CORE, mmdt=MMDT_DEFAULT, bufs=None):
    MMDT = F32R if mmdt == "f32r" else F32
    ntiles = b_core // TN
    bu = {"inp": 4, "work": 4, "scal": 4, "zsb": 9, "z2sb": 3, "osb": 4,
          "psm": 3, "psz": 2, "psf": 1, "psb2": 2}
    if bufs:
        bu.update(bufs)
    nc = bacc.Bacc("TRN2", target_bir_lowering=False, debug=False,
                   num_devices=NCORES)

    # ---------------- DRAM I/O ----------------
    d_x = nc.dram_tensor("x", [ntiles, IN_F, TN], MMDT, kind="ExternalInput")
    d_u = nc.dram_tensor("u", [ntiles, UDIM, TN], MMDT, kind="ExternalInput")
    d_out = nc.dram_tensor("out", [ntiles, NCH, 128, NCLS], F32, kind="ExternalOutput")

    def cin(name, shape, dt=F32):
        return nc.dram_tensor(name, shape, dt, kind="ExternalInput")

    d_ident = cin("ident", [128, 128])
    d_wbb1 = cin("wbb1", [IN_F, EMB], MMDT)
    d_wbb2 = cin("wbb2", [EMB, EMB], MMDT)
    d_b1 = cin("b1c", [EMB, 1])
    d_b2 = cin("b2c", [EMB, 1])
    d_beta = cin("betac", [EMB, 1])
    d_stat64 = cin("stat64", [128, 32])
    d_stl = cin("st_lhs", [2, 128], MMDT)
    d_wgU = cin("wgU", [EMB, 128], MMDT)
    d_wgV = cin("wgV", [UDIM, 128], MMDT)
    d_gsum = cin("gsum_lhs", [128, E], MMDT)
    d_gb = cin("gb_col", [E, 1])
    d_we1 = cin("we1", [128, 4, 128], MMDT)
    d_eb1 = cin("eb1", [128, 8])
    d_we2 = cin("we2", [128, 8, 32])
    d_wsb = cin("wsb_lhs", [48, 2, 128], MMDT)
    d_msum = cin("msum_lhs", [128, NCLS], MMDT)
    d_gw2c = cin("gw2c_lhs", [2 * E, NCLS], MMDT)

    with tile.TileContext(nc) as tc, ExitStack() as ctx:
        cpool = ctx.enter_context(tc.tile_pool(name="consts", bufs=1))
        p_in = ctx.enter_context(tc.tile_pool(name="inp", bufs=bu["inp"]))
        p_w = ctx.enter_context(tc.tile_pool(name="work", bufs=bu["work"]))
        p_sc = ctx.enter_context(tc.tile_pool(name="scal", bufs=bu["scal"]))
        p_z = ctx.enter_context(tc.tile_pool(name="zsb", bufs=bu["zsb"]))
        p_z2 = ctx.enter_context(tc.tile_pool(name="z2sb", bufs=bu["z2sb"]))
        p_out = ctx.enter_context(tc.tile_pool(name="osb", bufs=bu["osb"]))
        ps_m = ctx.enter_context(tc.tile_pool(name="psm", bufs=bu["psm"], space="PSUM"))
        ps_z = ctx.enter_context(tc.tile_pool(name="psz", bufs=bu["psz"], space="PSUM"))
        ps_f = ctx.enter_context(tc.tile_pool(name="psf", bufs=bu["psf"], space="PSUM"))
        ps_b = ctx.enter_context(tc.tile_pool(name="psb2", bufs=bu["psb2"], space="PSUM")) \
            if bu.get("psb2") else ps_m

        # ---------------- constants to SBUF ----------------
        c = {}
        for name, d, shape, dt in [
            ("ident", d_ident, [128, 128], F32),
            ("wbb1", d_wbb1, [IN_F, EMB], MMDT),
            ("wbb2", d_wbb2, [EMB, EMB], MMDT),
            ("b1", d_b1, [EMB, 1], F32),
            ("b2", d_b2, [EMB, 1], F32),
            ("beta", d_beta, [EMB, 1], F32),
            ("stat64", d_stat64, [128, 32], F32),
            ("stl", d_stl, [2, 128], MMDT),
            ("wgU", d_wgU, [EMB, 128], MMDT),
            ("wgV", d_wgV, [UDIM, 128], MMDT),
            ("gsum", d_gsum, [128, E], MMDT),
            ("gb", d_gb, [E, 1], F32),
            ("we1", d_we1, [128, 4, 128], MMDT),
            ("eb1", d_eb1, [128, 8], F32),
            ("we2", d_we2, [128, 8, 32], F32),
            ("wsb", d_wsb, [48, 2, 128], MMDT),
            ("msum", d_msum, [128, NCLS], MMDT),
            ("gw2c", d_gw2c, [2 * E, NCLS], MMDT),
        ]:
            t = cpool.tile(shape, dt, tag=name)
            nc.sync.dma_start(t[:], d.ap())
            c[name] = t

        ident = c["ident"]

        def tile_body(it):
            # ---------- load x / u (feature-major, host-prepped) ----------
            x_fm = p_in.tile([IN_F, TN], MMDT, tag="x_fm")
            nc.sync.dma_start(x_fm[:], d_x.ap()[it])
            u_fm = p_in.tile([UDIM, TN], MMDT, tag="u_fm")
            nc.sync.dma_start(u_fm[:], d_u.ap()[it])

            # ---------- backbone ----------
            ps1 = ps_m.tile([EMB, TN], F32, tag="psm")
            nc.tensor.matmul(ps1[:], c["wbb1"][:], x_fm[:], start=True, stop=True)
            h1 = p_w.tile([EMB, TN], MMDT, tag="h1")
            nc.scalar.activation(h1[:], ps1[:], AF.Gelu, bias=c["b1"][:])

            ps2 = ps_m.tile([EMB, TN], F32, tag="psm")
            nc.tensor.matmul(ps2[:], c["wbb2"][:], h1[:], start=True, stop=True)
            h2s = p_w.tile([128, TN], F32, tag="h2s")   # rows 0-63 h2, 64-127 h2^2
            nc.scalar.activation(h2s[0:EMB, :], ps2[:], AF.Gelu, bias=c["b2"][:])
            nc.scalar.activation(h2s[EMB:128, :], h2s[0:EMB, :], AF.Square)

            psb = ps_m.tile([2, TN], F32, tag="psm")     # mean(h2), mean(h2^2)
            nc.tensor.matmul(psb[:], c["stat64"][:, 0:2], h2s[:], start=True, stop=True)
            stats_bb = p_sc.tile([2, TN], F32, tag="stats_bb")
            nc.vector.tensor_copy(stats_bb[:], psb[:])

            # ---------- pass A: bb LN scalars (batch-major) ----------
            psA = ps_m.tile([128, NCH, 2], F32, tag="psm")
            for ch in range(NCH):
                nc.tensor.transpose(psA[:, ch, :], stats_bb[:, 128 * ch:128 * (ch + 1)],
                                    ident[0:2, 0:2])
            # var = (m2 + eps) - mu^2 ; rs = rsqrt(var) ; p = mu*rs
            sA = p_sc.tile([128, NCH, 2], F32, tag="sA")
            nc.vector.tensor_copy(sA[:], psA[:])
            tmpA = p_sc.tile([128, NCH], F32, tag="tmpA")
            nc.vector.tensor_tensor(tmpA[:], sA[:, :, 0], sA[:, :, 0], op=ALU.mult)
            vA = p_sc.tile([128, NCH], F32, tag="vA")
            nc.vector.scalar_tensor_tensor(vA[:], sA[:, :, 1], EPS_LN, tmpA[:],
                                           op0=ALU.add, op1=ALU.subtract)
            backA = p_sc.tile([128, NCH, 2], F32, tag="backA")
            rsA = backA[:, :, 0]
            _newton_rsqrt(nc, p_sc, vA[:], rsA, [128, NCH], "nA")
            nc.vector.tensor_tensor(backA[:, :, 1], rsA, sA[:, :, 0], op=ALU.mult)

            psBA = ps_m.tile([2, TN], F32, tag="psm")
            for ch in range(NCH):
                nc.tensor.transpose(psBA[:, 128 * ch:128 * (ch + 1)],
                                    backA[:, ch, :], ident[:])
            stf = p_sc.tile([2, TN], MMDT, tag="stf")
            nc.vector.tensor_copy(stf[:], psBA[:])

            # ---------- h = h2*S + (beta + T') ----------
            stp = ps_m.tile([128, TN], F32, tag="psm")
            nc.tensor.matmul(stp[:], c["stl"][:], stf[:], start=True, stop=True)
            tmph = p_w.tile([EMB, TN], F32, tag="tmph")
            nc.vector.tensor_tensor(tmph[:], h2s[0:EMB, :], stp[0:EMB, :], op=ALU.mult)
            h_fm = p_w.tile([128, TN], MMDT, tag="h_fm")
            nc.vector.scalar_tensor_tensor(h_fm[0:EMB, :], tmph[:], c["beta"][:],
                                           stp[EMB:128, :], op0=ALU.add, op1=ALU.add)
            nc.vector.tensor_copy(h_fm[EMB:128, :], h_fm[0:EMB, :])

            # ---------- gate ----------
            psU = ps_m.tile([128, TN], F32, tag="psm")
            nc.tensor.matmul(psU[:], c["wgU"][:], h_fm[0:EMB, :], start=True, stop=True)
            psV = ps_m.tile([128, TN], F32, tag="psm")
            nc.tensor.matmul(psV[:], c["wgV"][:], u_fm[:], start=True, stop=True)
            uVs = p_w.tile([128, TN], F32, tag="uVs")
            nc.scalar.copy(uVs[:], psV[:])
            gprod = p_w.tile([128, TN], MMDT, tag="gprod")
            nc.vector.tensor_tensor(gprod[:], psU[:], uVs[:], op=ALU.mult)
            psg = ps_m.tile([E, TN], F32, tag="psm")
            nc.tensor.matmul(psg[:], c["gsum"][:], gprod[:], start=True, stop=True)

            # ---------- experts fc1 (+gelu), z^2 ----------
            z_sb = []
            for q in range(4):
                zqA = ps_z.tile([128, TN], F32, tag=("psm" if bu.get("one_psum") else "zps"), name=f"zqA_{it}_{q}")
                zqB = ps_z.tile([128, TN], F32, tag=("psm" if bu.get("one_psum") else "zps"), name=f"zqB_{it}_{q}")
                nc.tensor.matmul(zqA[:], c["we1"][0:EMB, q, :], h_fm[0:EMB, :],
                                 start=True, stop=True, tile_position=(0, 0))
                nc.tensor.matmul(zqB[:], c["we1"][EMB:128, q, :],
                                 h_fm[EMB:128, :], start=True, stop=True,
                                 tile_position=(EMB, 0))
                for s, zq in enumerate((zqA, zqB)):
                    p = 2 * q + s
                    z = p_z.tile([128, TN], F32, tag="z_sb", name=f"z_{it}_{p}")
                    nc.scalar.activation(z[:], zq[:], AF.Gelu,
                                         bias=c["eb1"][:, p:p + 1])
                    z_sb.append(z)

            z2_sb = []
            for p in range(8):
                z2 = p_z2.tile([128, TN], F32, tag="z2_sb")
                eng = nc.gpsimd if p < 6 else nc.vector
                eng.tensor_tensor(z2[:], z_sb[p][:], z_sb[p][:], op=ALU.mult)
                z2_sb.append(z2)

            # ---------- expert stats (z^2) and fc2 (+mu), col-tiled ----------
            zst = [ps_m.tile([128, TN], F32, tag="psm", name=f"zst{it}_{i}") for i in range(2)]
            for grp in range(2):
                for j in range(4):
                    p = 4 * grp + j
                    nc.tensor.matmul(zst[grp][32 * j:32 * j + 32, :], c["stat64"][:],
                                     z2_sb[p][:], start=True, stop=True,
                                     tile_position=(0, 32 * j))
            fc2 = [ps_f.tile([128, TN], F32, tag=("psm" if bu.get("one_psum") else "fc2"), name=f"fc2_{it}_{i}") for i in range(2)]
            for grp in range(2):
                for j in range(4):
                    p = 4 * grp + j
                    nc.tensor.matmul(fc2[grp][32 * j:32 * j + 32, :],
                                     c["we2"][:, p, :], z_sb[p][:],
                                     start=True, stop=True, tile_position=(0, 32 * j))

            # ---------- stats to batch-major via full-bank transposes ----------
            # copy fc2 / zst psum banks to SBUF (fc2sb also feeds combine)
            fc2sb, zstsb = [], []
            for b in range(2):
                t = p_w.tile([128, TN], F32, tag="fc2sb", name=f"fc2sb_{it}_{b}")
                nc.scalar.copy(t[:], fc2[b][:])
                fc2sb.append(t)
                t2 = p_w.tile([128, TN], F32, tag="zstsb", name=f"zstsb_{it}_{b}")
                nc.scalar.copy(t2[:], zst[b][:])
                zstsb.append(t2)
            g_sb = p_sc.tile([E, TN], F32, tag="g_sb")
            nc.vector.tensor_scalar(g_sb[:], psg[:], c["gb"][:], None, op0=ALU.add)

            yield  # ---- frontend/backend pipeline split ----

            muB = p_sc.tile([128, NCH, E], F32, tag="muB")
            m2B = p_sc.tile([128, NCH, E], F32, tag="m2B")

            def _extract(src_ps, dst, base):
                sap = src_ps[:, :, 0]
                a = sap.ap
                sap2 = bass.AP(tensor=sap.tensor, offset=sap.offset + base,
                               ap=[a[0], a[1], [32, 4], [1, 2]])
                dap = dst.ap
                dst2 = bass.AP(tensor=dst.tensor, offset=dst.offset,
                               ap=[dap[0], dap[1], [2, 4], [1, 2]])
                nc.vector.tensor_copy(dst2, sap2)

            for b in range(2):
                psT = ps_b.tile([128, NCH, 128], F32, tag="psb2", name=f"psTf_{it}_{b}")
                for ch in range(NCH):
                    nc.tensor.transpose(psT[:, ch, :],
                                        fc2sb[b][:, 128 * ch:128 * (ch + 1)], ident[:])
                _extract(psT, muB[:, :, 8 * b:8 * b + 8], 20)
            for b in range(2):
                psT = ps_b.tile([128, NCH, 128], F32, tag="psb2", name=f"psTz_{it}_{b}")
                for ch in range(NCH):
                    nc.tensor.transpose(psT[:, ch, :],
                                        zstsb[b][:, 128 * ch:128 * (ch + 1)], ident[:])
                _extract(psT, m2B[:, :, 8 * b:8 * b + 8], 0)

            psTg = ps_b.tile([128, NCH, E], F32, tag="psb2", name=f"psTg_{it}")
            for ch in range(NCH):
                nc.tensor.transpose(psTg[:, ch, :], g_sb[:, 128 * ch:128 * (ch + 1)],
                                    ident[0:E, 0:E])
            gcp = p_sc.tile([128, NCH, E], F32, tag="gcp")
            nc.scalar.copy(gcp[:], psTg[:])

            # ---------- pass B math ----------
            tmpB = p_sc.tile([128, NCH, E], F32, tag="tmpB")
            nc.vector.tensor_tensor(tmpB[:], muB[:], muB[:], op=ALU.mult)
            vB = p_sc.tile([128, NCH, E], F32, tag="vB")
            nc.vector.scalar_tensor_tensor(vB[:], m2B[:], EPS_LN, tmpB[:],
                                           op0=ALU.add, op1=ALU.subtract)
            rsB = p_sc.tile([128, NCH, E], F32, tag="rsB")
            _newton_rsqrt(nc, p_sc, vB[:], rsB[:], [128, NCH, E], "nB")
            vm8 = p_sc.tile([128, NCH, 8], F32, tag="vm8")
            for ch in range(NCH):
                nc.vector.max(vm8[:, ch, :], gcp[:, ch, :])
            dg = p_sc.tile([128, NCH], F32, tag="dg")
            nc.vector.tensor_tensor(dg[:], vm8[:, :, 0], vm8[:, :, 1], op=ALU.subtract)
            th = p_sc.tile([128, NCH], F32, tag="th")
            nc.scalar.activation(th[:], dg[:], AF.Tanh, scale=0.5)
            w12 = p_sc.tile([128, NCH, 2], F32, tag="w12")
            nc.vector.tensor_scalar(w12[:, :, 0], th[:], 0.5, 0.5, op0=ALU.mult, op1=ALU.add)
            nc.vector.tensor_scalar(w12[:, :, 1], th[:], -0.5, 0.5, op0=ALU.mult, op1=ALU.add)

            is1 = p_sc.tile([128, NCH, E], F32, tag="is1")
            nc.vector.tensor_tensor(is1[:], gcp[:], _bc(vm8[:, :, 0:1], E), op=ALU.is_equal)
            is2 = p_sc.tile([128, NCH, E], F32, tag="is2")
            nc.vector.tensor_tensor(is2[:], gcp[:], _bc(vm8[:, :, 1:2], E), op=ALU.is_equal)
            w1t = p_sc.tile([128, NCH, E], F32, tag="w1t")
            nc.vector.tensor_tensor(w1t[:], is1[:], _bc(w12[:, :, 0:1], E), op=ALU.mult)
            w2t = p_sc.tile([128, NCH, E], F32, tag="w2t")
            nc.vector.tensor_tensor(w2t[:], is2[:], _bc(w12[:, :, 1:2], E), op=ALU.mult)

            # back block: cols 0-15 wsm, 16-31 w, 32-47 ws, 48-63 pad
            backB = p_sc.tile([128, NCH, 64], F32, tag="backB")
            nc.gpsimd.memset(backB[:].rearrange("p c k -> p (c k)"), 0.0)
            nc.vector.tensor_tensor(backB[:, :, 16:32], w1t[:], w2t[:], op=ALU.add)
            nc.vector.tensor_tensor(backB[:, :, 32:48], backB[:, :, 16:32], rsB[:], op=ALU.mult)
            nc.vector.tensor_tensor(backB[:, :, 0:16], backB[:, :, 32:48], muB[:],
                                    op=ALU.mult)

            # 2 transposes of [128,128] (chunk-pairs, 64-padded); cf de-interleaves
            psBB = ps_b.tile([128, 2, 128], F32, tag="psb2")
            backBv = backB[:].rearrange("p c k -> p (c k)")
            for hh in range(2):
                nc.tensor.transpose(psBB[:, hh, :],
                                    backBv[:, 128 * hh:128 * (hh + 1)], ident[:])
            cf = p_sc.tile([48, TN], MMDT, tag="cf")
            cfv = cf[:].rearrange("p (h c q) -> p h c q", h=2, c=2, q=128)
            nc.vector.tensor_copy(cfv[:, :, 0, :], psBB[0:48, :, :])
            nc.vector.tensor_copy(cfv[:, :, 1, :], psBB[64:112, :, :])

            # ---------- combine ----------
            lg = ps_b.tile([NCLS, TN], F32, tag="psb2")
            prods = []
            for b in range(2):
                wsr = ps_b.tile([128, TN], F32, tag="psb2")
                nc.tensor.matmul(wsr[:], c["wsb"][32:48, b, :], cf[32:48, :],
                                 start=True, stop=True)
                prod = p_w.tile([128, TN], F32, tag="prod", name=f"prod_{it}_{b}")
                nc.vector.tensor_tensor(prod[:], fc2sb[b][:], wsr[:], op=ALU.mult)
                prods.append(prod)
            psum2 = p_w.tile([128, TN], MMDT, tag="psum2")
            nc.vector.tensor_tensor(psum2[:], prods[0][:], prods[1][:], op=ALU.add)
            nc.tensor.matmul(lg[:], c["msum"][:], psum2[:], start=True, stop=False)
            nc.tensor.matmul(lg[:], c["gw2c"][:], cf[0:32, :], start=False, stop=True)

            lsb = p_out.tile([NCLS, TN], F32, tag="lsb")
            nc.scalar.copy(lsb[:], lg[:])
            psL = ps_b.tile([128, NCH * NCLS], F32, tag="psb2")
            for ch in range(NCH):
                nc.tensor.transpose(psL[:, NCLS * ch:NCLS * (ch + 1)],
                                    lsb[:, 128 * ch:128 * (ch + 1)],
                                    ident[0:NCLS, 0:NCLS])
            osb = p_out.tile([128, NCH, NCLS], F32, tag="osb")
            nc.vector.tensor_copy(osb[:], psL[:])
            nc.sync.dma_start(d_out.ap()[it].rearrange("c p k -> p c k"), osb[:])

        SKEW = int(os.environ.get("KSKEW", "1"))
        gens = []
        for it in range(ntiles):
            gen = tile_body(it)
            next(gen)
            gens.append(gen)
            if it >= SKEW:
                for _ in gens[it - SKEW]:
                    pass
        for it in range(max(0, ntiles - SKEW), ntiles):
            for _ in gens[it]:
                pass

    nc.compile()
    return nc


def _newton_rsqrt(nc, pool, v_ap, out_ap, shape, tag, eng=None):
    """out = 1/sqrt(v) via quake seed + Newton iterations."""
    eng = eng or nc.vector
    r = pool.tile(shape, F32, tag=tag + "_r")
    t = pool.tile(shape, F32, tag=tag + "_t")
    eng.tensor_scalar(r[:].bitcast(I32), v_ap.bitcast(I32), 1, None,
                      op0=ALU.logical_shift_right)
    eng.tensor_scalar(r[:].bitcast(I32), r[:].bitcast(I32), -1, 0x5F3759DF,
                      op0=ALU.mult, op1=ALU.add)
    niter = int(os.environ.get("KNEWTON", "2"))
    for i in range(niter):
        dst = out_ap if i == niter - 1 else r[:]
        eng.tensor_tensor(t[:], r[:], r[:], op=ALU.mult)
        eng.tensor_tensor(t[:], t[:], v_ap, op=ALU.mult)
        eng.tensor_scalar(t[:], t[:], -0.5, 1.5, op0=ALU.mult, op1=ALU.add)
        eng.tensor_tensor(dst, r[:], t[:], op=ALU.mult)


# ---------------------------------------------------------------------------
# host-side weight prep
# ---------------------------------------------------------------------------
def prep_consts(inp):
    f = np.float32
    gU, gV, gb = inp["gU"].astype(f), inp["gV"].astype(f), inp["gb"].astype(f)
    e_w1, e_b1 = inp["e_w1"].astype(f), inp["e_b1"].astype(f)
    e_g, e_beta = inp["e_g"].astype(f), inp["e_beta"].astype(f)
    e_w2, e_b2 = inp["e_w2"].astype(f), inp["e_b2"].astype(f)
    ut = inp["ut"].astype(f)
    bb_g, bb_beta = inp["bb_g"].astype(f), inp["bb_beta"].astype(f)

    cns = {}
    cns["ident"] = np.eye(128, dtype=f)
    cns["wbb1"] = inp["bb_w1"].astype(f)
    cns["wbb2"] = inp["bb_w2"].astype(f)
    cns["b1c"] = inp["bb_b1"].astype(f).reshape(EMB, 1)
    cns["b2c"] = inp["bb_b2"].astype(f).reshape(EMB, 1)
    cns["betac"] = bb_beta.reshape(EMB, 1)

    st = np.zeros((128, 32), f)
    st[0:64, 0] = 1.0 / 64
    st[64:128, 1] = 1.0 / 64
    cns["stat64"] = st

    stl = np.zeros((2, 128), f)
    stl[0, 0:64] = bb_g
    stl[1, 64:128] = -bb_g
    cns["st_lhs"] = stl

    wgU = np.zeros((EMB, 128), f)
    wgV = np.zeros((UDIM, 128), f)
    for e in range(E):
        wgU[:, e * RANK:(e + 1) * RANK] = gU[e]
        wgV[:, e * RANK:(e + 1) * RANK] = gV[e]
    cns["wgU"] = wgU
    cns["wgV"] = wgV

    gs = np.zeros((128, E), f)
    for i, e in enumerate(PERM):
        gs[e * RANK:(e + 1) * RANK, i] = 1.0
    cns["gsum_lhs"] = gs
    cns["gb_col"] = gb[PERM].reshape(E, 1)

    we1 = np.zeros((128, 4, 128), f)
    eb1 = np.zeros((128, 8), f)
    for q in range(4):
        # row-tile A (partitions 0-63) computes pair 2q, tile B pair 2q+1
        we1[0:64, q, :] = np.concatenate([e_w1[4 * q], e_w1[4 * q + 1]], axis=1)
        we1[64:128, q, :] = np.concatenate([e_w1[4 * q + 2], e_w1[4 * q + 3]], axis=1)
    for p in range(8):
        eb1[0:64, p] = e_b1[2 * p]
        eb1[64:128, p] = e_b1[2 * p + 1]
    cns["we1"] = we1
    cns["eb1"] = eb1

    we2 = np.zeros((128, 8, 32), f)
    for p in range(8):
        e0, e1 = 2 * p, 2 * p + 1
        we2[0:64, p, 0:10] = e_g[e0][:, None] * e_w2[e0]
        we2[64:128, p, 10:20] = e_g[e1][:, None] * e_w2[e1]
        we2[0:64, p, 20] = 1.0 / 64
        we2[64:128, p, 21] = 1.0 / 64
    cns["we2"] = we2

    wsb = np.zeros((48, 2, 128), f)
    for i, e in enumerate(PERM):
        p, q = e // 2, e % 2
        b, j = p // 4, p % 4
        wsb[32 + i, b, 32 * j + 10 * q:32 * j + 10 * q + 10] = 1.0
    cns["wsb_lhs"] = wsb

    ms = np.zeros((128, NCLS), f)
    for j in range(4):
        for q in range(2):
            for cc in range(NCLS):
                ms[32 * j + 10 * q + cc, cc] = 1.0
    cns["msum_lhs"] = ms

    gw2 = np.einsum("ed,edc->ec", e_g, e_w2)
    cst = np.einsum("ed,edc->ec", e_beta, e_w2) + e_b2
    gw2c = np.zeros((2 * E, NCLS), f)
    gw2c[0:E] = -gw2[PERM]
    gw2c[E:2 * E] = cst[PERM]
    cns["gw2c_lhs"] = gw2c

    return cns


def shard_inputs(x, user_ids, ut, b_core):
    """x [B,80] -> per-core [nt,80,512] feature-major; u gathered+transposed."""
    ncores = x.shape[0] // b_core
    nt = b_core // TN
    xs = np.ascontiguousarray(
        x.reshape(ncores, nt, TN, IN_F).transpose(0, 1, 3, 2).astype(np.float32))
    u = ut.astype(np.float32)[user_ids]          # [B, 16]
    us = np.ascontiguousarray(
        u.reshape(ncores, nt, TN, UDIM).transpose(0, 1, 3, 2))
    return xs, us


_CACHE = {}


def _get_program(b_core, mmdt):
    key = (b_core, mmdt)
    if key not in _CACHE:
        _CACHE[key] = build_program(b_core, mmdt)
    return _CACHE[key]


def kernel(**inputs):
    from concourse.bass_utils import run_bass_kernel_spmd
    mmdt = os.environ.get("KMMDT", MMDT_DEFAULT)
    x = np.asarray(inputs["x"], np.float32).reshape(B, IN_F)
    uids = np.asarray(inputs["user_ids"]).astype(np.int64)
    nc = _get_program(B_CORE, mmdt)
    cns = prep_consts({k: np.asarray(v) for k, v in inputs.items()})
    xs, us = shard_inputs(x, uids, np.asarray(inputs["ut"]), B_CORE)
    in_maps = []
    for k in range(NCORES):
        m = dict(cns)
        m["x"] = xs[k]
        m["u"] = us[k]
        in_maps.append(m)
    res = run_bass_kernel_spmd(nc, in_maps, core_ids=list(range(NCORES)))
    out = np.concatenate([r["out"].reshape(B_CORE, NCLS) for r in res.results], axis=0)
    return out.astype(np.float32)
